# revision 104
# baseline (speedup 1.0000x reference)
"""Sharded attention kernel for Trainium2 (8 NeuronCores, Bass/Tile).

Module: x->(wq,wk,wv) qk-norm + rope + GQA self-attn  (+)  gated cross-attn
over y->(wk_y,wv_y), then wo.  B=2, S=2048, D=2048, H=16, KV=8, HD=128,
YL=256, YD=1024.

Sharding: 2-way batch DP x 4-way head TP.  Core c handles batch c//4 and
head group g=c%4 (q heads 4g..4g+3, kv heads 2g..2g+1, y-heads
(4g%8)..(4g%8)+3).  wo is row-sharded; the 4 partial outputs per batch are
summed on the host.  The q/k/ky layernorms normalize over the *full* flat
head dim, so each core computes partial (sum, sumsq) stats and three small
in-kernel AllReduces (groups [[0..3],[4..7]]) produce the full-row moments.

Projections run as error-compensated fp8 (e4m3) DoubleRow matmuls: the host
splits x / y / every projection weight into hi+lo fp8 streams (same DMA
bytes as fp16) with fixed power-of-2 scales; on device each contraction
pair-chunk issues three DoubleRow matmuls (hh, hl, lh) into the same fp32
PSUM accumulation, recovering ~2^-8 effective operand precision at 3/4 the
fp16 PE cost.  Evictions fold the descale into the activation-copy scale.
Attention stays fp16: scores are computed transposed (keys on partitions)
so the key mask folds into the exp() bias and P feeds PV untransposed;
softmax skips max-subtraction; denominators accumulate on DVE and reduce
across partitions with a gpsimd partition_all_reduce (Pool engine), keeping
the softmax tail entirely off PE and PSUM.  wo is interleaved into the
second query-block's attention to fill PE while Activation runs exp.
"""
import sys

sys.path.insert(0, "/opt/trn_rl_repo")

import numpy as np
import ml_dtypes

import concourse.bass as bass  # noqa: F401
import concourse.tile as tile
from concourse import bacc, mybir, bass_isa
from concourse import bass_utils
from concourse.masks import make_identity

BF16 = mybir.dt.bfloat16
DT16 = mybir.dt.float16
F32 = mybir.dt.float32
FP8 = mybir.dt.float8e4
NPFP8 = mybir.dt.np(FP8)
NP16 = np.float16
DR = mybir.MatmulPerfMode.DoubleRow

B, S, D, H, KV, YL, YD, HD = 2, 2048, 2048, 16, 8, 256, 1024, 128
N_CORES, TP = 8, 4
HPC, KVPC, YHPC = 4, 2, 4          # q / kv / y heads per core
QW, KW, YW = HPC * HD, KVPC * HD, YHPC * HD   # 512, 256, 512 output cols
NDC, NYC = D // 128, YD // 128     # contraction chunks: 16, 8
NSB, SB = 4, 512                   # seq blocks for projections
NQB, QB = 2, 1024                  # query blocks for attention
JB = 512                           # attention j-chunk (max moving free)
NKC = S // 128                     # 16 key chunks (self)
NYKC = YL // 128                   # 2 key chunks (cross)
NST = S // 128                     # 16 seq tiles for wo
EPS_QK, EPS_KY = 1e-5, 1e-6
NEG = -1.0e30
XS, WS = 32.0, 2048.0              # fixed fp8 scales (power of 2)
DESC = 1.0 / (XS * WS)             # eviction descale

_RUNNER = None
_EXEC = None


def _build_program(use_cc=True):
    nc = bacc.Bacc("TRN2", target_bir_lowering=False, debug=False,
                   num_devices=N_CORES if use_cc else 1)

    def din(name, shape, dt=DT16):
        return nc.dram_tensor(name, shape, dt, kind="ExternalInput")

    t = dict(
        xhl=din("xhl", [D, 2, S], FP8),
        yhl=din("yhl", [YD, 2, YL], FP8),
        wqhl=din("wqhl", [D, 2, QW], FP8),
        wkvhl=din("wkvhl", [D, 2, 2 * KW], FP8),
        wyhl=din("wyhl", [YD, 2, 2 * YW], FP8),
        wo=din("wo", [QW, D]),
        CC=din("CC", [128, S]),
        SSp=din("SSp", [128, S]),
        swapP=din("swapP", [128, 128]),
        qgc=din("qgc", [128, HPC], F32),
        kgc=din("kgc", [128, KVPC], F32),
        kygc=din("kygc", [128, YHPC], F32),
        qb=din("qb", [128, HPC], F32),
        kb=din("kb", [128, KVPC], F32),
        kyb=din("kyb", [128, YHPC], F32),
        xmask=din("xmask", [128, NKC], F32),
        ymask=din("ymask", [128, NYKC], F32),
        tg=din("tg", [128, YHPC], F32),
        out=nc.dram_tensor("out", [S, D], DT16, kind="ExternalOutput"),
        kin=nc.dram_tensor("kin", [2, S], F32),
        kout=nc.dram_tensor("kout", [2, S], F32),
        kyin=nc.dram_tensor("kyin", [2, YL], F32),
        kyout=nc.dram_tensor("kyout", [2, YL], F32),
        qin=nc.dram_tensor("qin", [2, S], F32),
        qout=nc.dram_tensor("qout", [2, S], F32),
        lnr=nc.dram_tensor("lnr", [6, S], DT16),
        groups=[[0, 1, 2, 3], [4, 5, 6, 7]],
        use_cc=use_cc,
    )

    with tile.TileContext(nc) as tc:
        _emit(nc, tc, t)
    nc.compile()
    return nc


def _allreduce(nc, t, name, nh, h):
    """AllReduce of one half-major block of the partial LN stats (cc) /
    local copy (no-cc).  The stats tensors are laid out half-major:
    block h holds [sum_cols | sq_cols] for its column range contiguously,
    so the collective input is a single contiguous run."""
    Alu = mybir.AluOpType
    tin, tout = t[name + "in"], t[name + "out"]
    n = 2 * tin.shape[1] // nh
    src = bass.AP(tensor=tin.ap().tensor, offset=h * n, ap=[[1, n]])
    dst = bass.AP(tensor=tout.ap().tensor, offset=h * n, ap=[[1, n]])
    if t["use_cc"]:
        nc.gpsimd.collective_compute(
            "AllReduce", Alu.add, replica_groups=t["groups"],
            ins=[src], outs=[dst])


def _st(t, name):
    """Stats source for moments: the AllReduce output when collectives
    run; the local partials directly in the single-core timing variant
    (the collective adds no local engine work there)."""
    return t[name + "out"] if t["use_cc"] else t[name + "in"]


def _emit(nc, tc, t):
    AF = mybir.ActivationFunctionType
    Alu = mybir.AluOpType

    cm_consts = tc.tile_pool(name="consts", bufs=1)
    consts = cm_consts.__enter__()

    # ---------------- constants / small inputs ----------------
    ident = consts.tile([128, 128], DT16, tag="ident", name="ident")
    make_identity(nc, ident[:, :])
    ones_col = consts.tile([128, 1], DT16, tag="ones_col", name="ones_col")
    nc.vector.memset(ones_col[:, :], 1.0)
    ones_bf = consts.tile([128, 1], BF16, tag="ones_bf", name="ones_bf")
    nc.vector.memset(ones_bf[:, :], 1.0)
    swp = consts.tile([128, 128], DT16, tag="swp", name="swp")
    cc = consts.tile([128, S], DT16, tag="cc", name="cc")
    ssp = consts.tile([128, S], DT16, tag="ssp", name="ssp")
    qg_sb = consts.tile([128, HPC], F32, tag="qgc", name="qgc")
    nc.gpsimd.dma_start(qg_sb[:, :], t["qgc"].ap())
    kg_sb = consts.tile([128, KVPC], F32, tag="kgc", name="kgc")
    nc.gpsimd.dma_start(kg_sb[:, :], t["kgc"].ap())
    kyg_sb = consts.tile([128, YHPC], F32, tag="kygc", name="kygc")
    nc.gpsimd.dma_start(kyg_sb[:, :], t["kygc"].ap())
    qb_sb = consts.tile([128, HPC], F32, tag="qb", name="qb")
    nc.gpsimd.dma_start(qb_sb[:, :], t["qb"].ap())
    kb_sb = consts.tile([128, KVPC], F32, tag="kb", name="kb")
    nc.gpsimd.dma_start(kb_sb[:, :], t["kb"].ap())
    kyb_sb = consts.tile([128, YHPC], F32, tag="kyb", name="kyb")
    nc.gpsimd.dma_start(kyb_sb[:, :], t["kyb"].ap())
    xm_sb = consts.tile([128, NKC], F32, tag="xm", name="xm")
    nc.gpsimd.dma_start(xm_sb[:, :], t["xmask"].ap())
    ym_sb = consts.tile([128, NYKC], F32, tag="ym", name="ym")
    nc.gpsimd.dma_start(ym_sb[:, :], t["ymask"].ap())
    tg_sb = consts.tile([128, YHPC], F32, tag="tg", name="tg")
    nc.gpsimd.dma_start(tg_sb[:, :], t["tg"].ap())

    # ---------------- phase-1 pools ----------------
    cm_raw = tc.tile_pool(name="p_raw", bufs=1)
    p_raw = cm_raw.__enter__()
    cm_w = tc.tile_pool(name="p_w", bufs=1)
    p_w = cm_w.__enter__()
    cm_x = tc.tile_pool(name="p_x", bufs=1)
    p_x = cm_x.__enter__()
    cm_wsq = tc.tile_pool(name="w_sq", bufs=2)
    w_sq = cm_wsq.__enter__()
    cm_stg = tc.tile_pool(name="w_stg", bufs=2)
    w_stg = cm_stg.__enter__()
    cm_ar = tc.tile_pool(name="w_ar", bufs=2)
    w_ar = cm_ar.__enter__()

    def stat_out(tname, nh, stg, col0, blk):
        """Write the accumulated (sum | sumsq) staging row with one DMA
        into the half-major stats layout."""
        hw_ = t[tname].shape[1] // nh          # cols per half
        h, rel = col0 // hw_, col0 % hw_
        dst = bass.AP(tensor=t[tname].ap().tensor, offset=2 * h * hw_ + rel,
                      ap=[[hw_, 2], [1, blk]])
        nc.scalar.dma_start(dst, stg[:, :2 * blk])

    # hi/lo-packed fp8 tiles: dim -2 selects the stream (0=hi, 1=lo)
    wq_sb = p_w.tile([128, NDC, 2, QW], FP8, tag="wq", name="wq")
    wkv_sb = p_w.tile([128, NDC, 2, 2 * KW], FP8, tag="wkv", name="wkv")
    wy_sb = p_w.tile([128, NYC, 2, 2 * YW], FP8, tag="wy", name="wy")
    y_sb = p_w.tile([128, NYC, 2, YL], FP8, tag="y", name="y")

    xr = t["xhl"].ap().rearrange("(c p) two s -> p c two s", p=128)
    wq_r = t["wqhl"].ap().rearrange("(c p) two m -> p c two m", p=128)
    wkv_r = t["wkvhl"].ap().rearrange("(c p) two m -> p c two m", p=128)
    wy_r = t["wyhl"].ap().rearrange("(c p) two m -> p c two m", p=128)
    y_r = t["yhl"].ap().rearrange("(c p) two s -> p c two s", p=128)

    # x tiles for all 4 seq blocks stay resident (both passes read them)
    xts = [p_x.tile([128, NDC, 2, SB], FP8, tag=f"x_{sb}", name=f"x_{sb}")
           for sb in range(NSB)]

    # load order: kv pass runs first, so wkv-hi + x0-hi strips lead (the
    # hh-term sweeps start on them), then the lo strips, then the rest.
    for s in (0, 1):
        for c in range(0, NDC, 4):
            nc.sync.dma_start(wkv_sb[:, c:c + 4, s, :],
                              wkv_r[:, c:c + 4, s, :])
            nc.sync.dma_start(xts[0][:, c:c + 4, s, :],
                              xr[:, c:c + 4, s, 0:SB])
    for sb in range(1, NSB):
        for c in range(0, NDC, 8):
            for s in (0, 1):
                nc.sync.dma_start(xts[sb][:, c:c + 8, s, :],
                                  xr[:, c:c + 8, s, sb * SB:(sb + 1) * SB])
    for c in range(0, NDC, 4):
        for s in (0, 1):
            nc.sync.dma_start(wq_sb[:, c:c + 4, s, :],
                              wq_r[:, c:c + 4, s, :])
    nc.sync.dma_start(swp[:, :], t["swapP"].ap())
    nc.sync.dma_start(cc[:, :], t["CC"].ap())
    nc.sync.dma_start(ssp[:, :], t["SSp"].ap())
    for s in (0, 1):
        nc.sync.dma_start(y_sb[:, :, s, :], y_r[:, :, s, :])
    for s in (0, 1):
        nc.sync.dma_start(wy_sb[:, :, s, :], wy_r[:, :, s, :])

    # raw projection outputs (fp16); later reused in place for QT/KT/vnat
    qraw = [p_raw.tile([128, S], DT16, tag=f"qraw{i}", name=f"qraw{i}")
            for i in range(HPC)]
    kraw = [p_raw.tile([128, S], DT16, tag=f"kraw{i}", name=f"kraw{i}")
            for i in range(KVPC)]
    vraw = [p_raw.tile([128, S], DT16, tag=f"vraw{i}", name=f"vraw{i}")
            for i in range(KVPC)]
    ykraw = [p_raw.tile([128, YL], DT16, tag=f"ykraw{i}", name=f"ykraw{i}")
             for i in range(YHPC)]
    yvraw = [p_raw.tile([128, YL], DT16, tag=f"yvraw{i}", name=f"yvraw{i}")
             for i in range(YHPC)]

    cm_psA = tc.tile_pool(name="pp_projA", bufs=2, space="PSUM")
    pp_proj = cm_psA.__enter__()

    def proj_fp8(w_t, x_t, npair, col0, blk, ps):
        """3-term compensated fp8 DoubleRow accumulation into ps.
        hh terms sweep first so compute can start before lo streams land."""
        first = True
        for (ws_, xs_) in ((0, 0), (0, 1), (1, 0)):
            for c in range(npair):
                nc.tensor.matmul(
                    ps[:, :blk],
                    w_t[:, 2 * c:2 * c + 2, ws_, col0:col0 + 128],
                    x_t[:, 2 * c:2 * c + 2, xs_, :blk],
                    start=first,
                    stop=((ws_, xs_) == (1, 0) and c == npair - 1),
                    perf_mode=DR)
                first = False

    def proj_block(w_t, x_t, npair, col0, dst, sb, blk,
                   stg=None, first=False):
        ps = pp_proj.tile([128, SB], F32, tag="proj", name="proj")
        proj_fp8(w_t, x_t, npair, col0, blk, ps)
        nc.scalar.activation(dst[:, sb * blk:(sb + 1) * blk], ps[:, :blk],
                             AF.Copy, scale=DESC)
        if stg is not None:
            # LN stats off PE: Pool partition-reduces the evicted tile and
            # its square; DVE accumulates the row into the staging tile
            sq = w_sq.tile([128, SB], BF16, tag="sqscratch", name="sqscratch")
            nc.scalar.activation(sq[:, :blk], ps[:, :blk], AF.Square,
                                 scale=DESC)
            ars = w_ar.tile([128, SB], DT16, tag="ars", name="ars")
            nc.gpsimd.partition_all_reduce(
                ars[:, :blk], dst[:, sb * blk:(sb + 1) * blk],
                channels=128, reduce_op=bass_isa.ReduceOp.add)
            arq = w_ar.tile([128, SB], BF16, tag="arq", name="arq")
            nc.gpsimd.partition_all_reduce(
                arq[:, :blk], sq[:, :blk],
                channels=128, reduce_op=bass_isa.ReduceOp.add)
            if first:
                nc.vector.tensor_copy(stg[0:1, :blk], ars[0:1, :blk])
                nc.vector.tensor_copy(stg[0:1, blk:2 * blk], arq[0:1, :blk])
            else:
                nc.vector.tensor_add(stg[0:1, :blk], stg[0:1, :blk],
                                     ars[0:1, :blk])
                nc.vector.tensor_add(stg[0:1, blk:2 * blk],
                                     stg[0:1, blk:2 * blk], arq[0:1, :blk])

    cm_rm = tc.tile_pool(name="rows_m", bufs=2, side="right")
    rows_m = cm_rm.__enter__()
    cm_wln = tc.tile_pool(name="w_ln", bufs=1, side="right")
    w_ln = cm_wln.__enter__()
    cm_wln2 = tc.tile_pool(name="w_ln2", bufs=2, side="right")
    w_ln2 = cm_wln2.__enter__()

    def moments(src_t, nh, n, inv_scale, eps, length, r_rstd, col0=0,
                ncols=None):
        """src_t is half-major (sum cols | sq cols per half); process the
        half starting at column col0.  Partition-parallel math on
        [128, ncols/128]; rstd and -mu*rstd slices land in lnr rows
        (r_rstd, r_rstd+1) via one DMA each way."""
        ncols = ncols or length
        J = ncols // 128
        hw_ = length // nh

        ab = rows_m.tile([128, 2, 16], F32, tag="mab", name="mab")
        src = bass.AP(tensor=src_t.ap().tensor,
                      offset=2 * (col0 // hw_) * hw_ + col0 % hw_,
                      ap=[[J, 128], [hw_, 2], [1, J]])
        nc.scalar.dma_start(ab[:, :, :J], src)
        a, b = ab[:, 0, :J], ab[:, 1, :J]
        nc.vector.tensor_scalar_mul(a, a, inv_scale / n)
        nc.vector.tensor_scalar_mul(b, b, inv_scale / n)
        c_ = rows_m.tile([128, 16], F32, tag="mc", name="mc")
        nc.vector.tensor_mul(c_[:, :J], a, a)
        nc.vector.tensor_tensor(b, b, c_[:, :J], Alu.subtract)
        nc.vector.tensor_scalar_add(b, b, eps)
        # rsqrt via exp(-0.5*ln(var)): stays in the exp activation table
        # (no table switch before attention); the Newton step below refines.
        nc.scalar.activation(c_[:, :J], b, AF.Ln)
        nc.scalar.activation(c_[:, :J], c_[:, :J], AF.Exp, scale=-0.5)
        d = rows_m.tile([128, 16], F32, tag="md", name="md")
        nc.vector.tensor_mul(d[:, :J], c_[:, :J], c_[:, :J])
        nc.vector.tensor_mul(d[:, :J], d[:, :J], b)
        nc.vector.tensor_scalar(out=d[:, :J], in0=d[:, :J],
                                scalar1=-0.5, scalar2=1.5,
                                op0=Alu.mult, op1=Alu.add)
        nc.vector.tensor_mul(c_[:, :J], c_[:, :J], d[:, :J])
        nc.vector.tensor_mul(a, a, c_[:, :J])
        nc.vector.tensor_scalar_mul(a, a, -1.0)
        ra = rows_m.tile([128, 2, 16], DT16, tag="mra", name="mra")
        nc.vector.tensor_copy(ra[:, 0, :J], c_[:, :J])
        nc.vector.tensor_copy(ra[:, 1, :J], a)
        out_r = bass.AP(tensor=t["lnr"].ap().tensor,
                        offset=r_rstd * S + col0,
                        ap=[[J, 128], [S, 2], [1, J]])
        nc.sync.dma_start(out_r, ra[:, :, :J])

    def dma_bcast(rgng, r_rstd, col0, ncols):
        """One DMA: broadcast lnr rows (r_rstd, r_rstd+1) column slice into
        the [128, 2, *] rgng tile."""
        src_ap = bass.AP(tensor=t["lnr"].ap().tensor,
                         offset=r_rstd * S + col0,
                         ap=[[0, 128], [S, 2], [1, ncols]])
        nc.sync.dma_start(rgng[:, :, col0:col0 + ncols], src_ap)

    cm_swp = tc.tile_pool(name="pp_swap", bufs=2, space="PSUM")
    pp_swap = cm_swp.__enter__()

    # broadcast-row tiles for the three LN streams ([:, 0, :]=rstd*,
    # [:, 1, :]=-mu*rstd)
    q_rr = w_ln.tile([128, 2, S], DT16, tag="q_rr", name="q_rr")
    k_rr = w_ln.tile([128, 2, S], DT16, tag="k_rr", name="k_rr")
    ky_rr = w_ln.tile([128, 2, YL], DT16, tag="ky_rr", name="ky_rr")

    def ln_unit(raw, rr, g_col, b_col, col0, ncols, rope, ps_pool=None,
                eng="dve"):
        """LayerNorm (+optional rope) of one head's column slice, in place.
        In phase 1 the per-head gain/bias and swap eviction run on
        Activation (per-partition scalars) to unload DVE at the boundary;
        units deferred into the attention region (ps_pool set) keep those
        ops on DVE so they don't compete with exp."""
        sl = slice(col0, col0 + ncols)
        in_attn = ps_pool is not None
        t1 = w_ln2.tile([128, SB], DT16, tag="lnt1", name="lnt1")
        nc.vector.tensor_mul(t1[:, :ncols], raw[:, sl], rr[:, 0, sl])
        nc.vector.tensor_add(t1[:, :ncols], t1[:, :ncols], rr[:, 1, sl])
        if not rope:
            if eng == "act":
                nc.scalar.activation(raw[:, sl], t1[:, :ncols], AF.Identity,
                                     bias=b_col, scale=g_col)
            else:
                nc.vector.tensor_scalar(out=raw[:, sl], in0=t1[:, :ncols],
                                        scalar1=g_col, scalar2=b_col,
                                        op0=Alu.mult, op1=Alu.add)
            return
        if eng == "act":
            nc.scalar.activation(t1[:, :ncols], t1[:, :ncols], AF.Identity,
                                 bias=b_col, scale=g_col)
        else:
            nc.vector.tensor_scalar(out=t1[:, :ncols], in0=t1[:, :ncols],
                                    scalar1=g_col, scalar2=b_col,
                                    op0=Alu.mult, op1=Alu.add)
        sw = w_ln2.tile([128, SB], DT16, tag="swap", name="swap")
        pool = ps_pool or pp_swap
        ps = pool.tile([128, JB], F32, tag="tp" if ps_pool is None else "po",
                       name="lnswp")
        nc.tensor.matmul(ps[:, :ncols], swp[:, :], t1[:, :ncols],
                         start=True, stop=True)
        if eng == "act":
            nc.scalar.activation(sw[:, :ncols], ps[:, :ncols], AF.Copy)
        else:
            nc.vector.tensor_copy(sw[:, :ncols], ps[:, :ncols])
        nc.vector.tensor_mul(t1[:, :ncols], t1[:, :ncols], cc[:, sl])
        nc.vector.tensor_mul(sw[:, :ncols], sw[:, :ncols], ssp[:, sl])
        nc.vector.tensor_add(raw[:, sl], t1[:, :ncols], sw[:, :ncols])

    def vtrans(raw, c):
        tp = pp_swap.tile([128, 128], DT16, tag="vtp", name="vtp")
        nc.tensor.transpose(tp[:, :], raw[:, c * 128:(c + 1) * 128],
                            ident[:, :])
        nc.scalar.activation(raw[:, c * 128:(c + 1) * 128], tp[:, :],
                             AF.Copy)

    def q_ln_unit(h, sb, ps_pool=None, eng="dve"):
        ln_unit(qraw[h], q_rr, qg_sb[:, h:h + 1],
                qb_sb[:, h:h + 1], sb * SB, SB, True, ps_pool, eng)

    def k_ln_sb(sb):
        for i in range(KVPC):
            ln_unit(kraw[i], k_rr, kg_sb[:, i:i + 1],
                    kb_sb[:, i:i + 1], sb * SB, SB, True)

    # ============ phase 1a: k/v projections + stats; AR in halves ==========
    for sb in range(NSB):
        xt = xts[sb]
        kstg = w_stg.tile([1, 2 * SB], F32, tag="stg", name="stg")
        for i in range(KVPC):
            proj_block(wkv_sb, xt, NDC // 2, i * 128, kraw[i], sb, SB,
                       kstg, first=(i == 0))
        for i in range(KVPC):
            proj_block(wkv_sb, xt, NDC // 2, KW + i * 128, vraw[i], sb, SB)
        stat_out("kin", 2, kstg, sb * SB, SB)
        if sb == 1:
            _allreduce(nc, t, "k", 2, 0)
        if sb == 2:
            moments(_st(t, "k"), 2, KV * HD, 1.0, EPS_QK, S, 2, 0, 2 * SB)
            dma_bcast(k_rr, 2, 0, 2 * SB)
        for i in range(KVPC):
            for c in range(4 * sb, 4 * sb + 4):
                vtrans(vraw[i], c)
        if sb == 3:
            k_ln_sb(0)
    k_ln_sb(1)
    _allreduce(nc, t, "k", 2, 1)
    moments(_st(t, "k"), 2, KV * HD, 1.0, EPS_QK, S, 2, 2 * SB, 2 * SB)
    dma_bcast(k_rr, 2, 2 * SB, 2 * SB)

    # ============ phase 1b: q projections; k-LN tail interleaved; the q
    # stats AR runs in halves so query-block-0's LN lands inside the pass ===
    for sb in range(NSB):
        xt = xts[sb]
        qstg = w_stg.tile([1, 2 * SB], F32, tag="stg", name="stg")
        for i in range(HPC):
            proj_block(wq_sb, xt, NDC // 2, i * 128, qraw[i], sb, SB,
                       qstg, first=(i == 0))
        stat_out("qin", 2, qstg, sb * SB, SB)
        if sb == 0:
            k_ln_sb(2)
        if sb == 1:
            _allreduce(nc, t, "q", 2, 0)
            k_ln_sb(3)
        if sb == 2:
            moments(_st(t, "q"), 2, H * HD, 1.0, EPS_QK, S, 0, 0, 2 * SB)
            dma_bcast(q_rr, 0, 0, 2 * SB)
        if sb == 3:
            for h in range(HPC):
                q_ln_unit(h, 0)
    for h in range(HPC):
        q_ln_unit(h, 1, eng=("act" if h % 2 == 0 else "dve"))
    _allreduce(nc, t, "q", 2, 1)
    moments(_st(t, "q"), 2, H * HD, 1.0, EPS_QK, S, 0, 2 * SB, 2 * SB)
    dma_bcast(q_rr, 0, 2 * SB, 2 * SB)

    # ---- y projections ----
    ystg = w_stg.tile([1, 2 * SB], F32, tag="stg", name="stg")
    for i in range(YHPC):
        proj_block(wy_sb, y_sb, NYC // 2, i * 128, ykraw[i], 0, YL,
                   ystg, first=(i == 0))
    for i in range(YHPC):
        proj_block(wy_sb, y_sb, NYC // 2, YW + i * 128, yvraw[i], 0, YL)
    stat_out("kyin", 1, ystg, 0, YL)

    _allreduce(nc, t, "ky", 1, 0)

    for i in range(YHPC):
        for c in range(NYKC):
            vtrans(yvraw[i], c)

    # ---- ky LN (no rope, no PE work) ----
    moments(_st(t, "ky"), 1, KV * HD, 0.5, EPS_KY, YL, 4)
    dma_bcast(ky_rr, 4, 0, YL)
    for i in range(YHPC):
        ln_unit(ykraw[i], ky_rr, kyg_sb[:, i:i + 1],
                kyb_sb[:, i:i + 1], 0, YL, False, eng="act")

    # q-LN for the query-block-1 slices is deferred into the qb0 attention
    # region (emitted inside the attention loop below)

    QT, KT, YKT = qraw, kraw, ykraw

    def vnat(i, c):
        return vraw[i][:, c * 128:(c + 1) * 128]

    def yvnat(i, c):
        return yvraw[i][:, c * 128:(c + 1) * 128]

    cm_swp.__exit__(None, None, None)
    cm_psA.__exit__(None, None, None)
    cm_ar.__exit__(None, None, None)
    cm_stg.__exit__(None, None, None)
    cm_wsq.__exit__(None, None, None)
    cm_x.__exit__(None, None, None)
    cm_w.__exit__(None, None, None)

    # ============ attention + wo ============
    cm_out = tc.tile_pool(name="p_out", bufs=1)
    p_out = cm_out.__enter__()
    outT = [p_out.tile([128, S], DT16, tag=f"outT{h}", name=f"outT{h}")
            for h in range(HPC)]
    cm_wo = tc.tile_pool(name="p_wo", bufs=1)
    p_wo = cm_wo.__enter__()
    wo_sb = p_wo.tile([128, HPC, D], DT16, tag="wo", name="wo")
    nc.sync.dma_start(wo_sb[:, :, :],
                      t["wo"].ap().rearrange("(c p) m -> p c m", p=128))
    cm_wat = tc.tile_pool(name="w_at", bufs=3)
    w_at = cm_wat.__enter__()
    cm_pt = tc.tile_pool(name="w_pt", bufs=7)
    w_pt = cm_pt.__enter__()
    cm_wob = tc.tile_pool(name="w_ob", bufs=2)
    w_ob = cm_wob.__enter__()

    cm_wops = tc.tile_pool(name="pp_wo", bufs=2, space="PSUM")
    cm_sc = tc.tile_pool(name="pp_sc", bufs=2, space="PSUM")
    cm_pv = tc.tile_pool(name="pp_pv", bufs=1, space="PSUM")
    pp_wo = cm_wops.__enter__()
    pp_sc = cm_sc.__enter__()
    pp_pv = cm_pv.__enter__()

    def attend(h, qb_i):
        """Self + gated cross attention for query block qb_i of head h."""
        q0 = qb_i * QB
        pv = pp_pv.tile([128, QB], F32, tag="pv", name="pv")

        def chunks(KT_h, vn, nkc, mask_sb, acc_tag):
            acc = w_at.tile([128, QB], DT16, tag=acc_tag, name=acc_tag)
            accB = (w_at.tile([128, QB], DT16, tag=acc_tag + "B",
                              name=acc_tag + "B") if nkc > 4 else None)
            ptA0 = ptB0 = None

            def emit_sc(c):
                sc = pp_sc.tile([128, QB], F32, tag="sc", name="sc")
                for j in range(0, QB, JB):
                    nc.tensor.matmul(sc[:, j:j + JB],
                                     KT_h[:, c * 128:(c + 1) * 128],
                                     QT[h][:, q0 + j:q0 + j + JB],
                                     start=True, stop=True)
                return sc

            # software pipeline: scores one chunk ahead so PE never
            # blocks behind exp(c) when issuing pv(c)
            sc_cur = emit_sc(0)
            for c in range(nkc):
                pt = w_pt.tile([128, QB], DT16, tag="ptile", name="ptile")
                nc.scalar.activation(pt[:, :], sc_cur[:, :], AF.Exp,
                                     bias=mask_sb[:, c:c + 1])
                if c + 1 < nkc:
                    sc_cur = emit_sc(c + 1)
                for j in range(0, QB, JB):
                    nc.tensor.matmul(pv[:, j:j + JB], vn(c),
                                     pt[:, j:j + JB],
                                     start=(c == 0), stop=(c == nkc - 1))
                # two parallel accumulation chains; each chain's first two
                # tiles fuse into one add (no initial copy)
                if accB is not None and c % 4 == 3 and c < nkc - 4:
                    if c == 3:
                        ptB0 = pt
                    elif ptB0 is not None:
                        nc.gpsimd.tensor_add(accB[:, :], ptB0[:, :], pt[:, :])
                        ptB0 = None
                    else:
                        nc.gpsimd.tensor_add(accB[:, :], accB[:, :],
                                             pt[:, :])
                elif c == 0:
                    ptA0 = pt
                elif ptA0 is not None:
                    nc.vector.tensor_add(acc[:, :], ptA0[:, :], pt[:, :])
                    ptA0 = None
                else:
                    nc.vector.tensor_add(acc[:, :], acc[:, :], pt[:, :])
            if accB is not None:
                nc.vector.tensor_add(acc[:, :], acc[:, :], accB[:, :])
            pvb = w_at.tile([128, QB], DT16, tag="pvb" + acc_tag,
                            name="pvb" + acc_tag)
            nc.vector.tensor_copy(pvb[:, :], pv[:, :])
            ar = w_at.tile([128, QB], DT16, tag="ar" + acc_tag,
                           name="ar" + acc_tag)
            nc.gpsimd.partition_all_reduce(ar[:, :], acc[:, :], channels=128,
                                           reduce_op=bass_isa.ReduceOp.add)
            with nc.allow_low_precision(reason="fp16 softmax denominators"):
                nc.vector.reciprocal(ar[:, :], ar[:, :])
            return pvb, ar

        pvbS, recS = chunks(KT[h // 2], lambda c: vnat(h // 2, c), NKC,
                            xm_sb, "S")
        pvbY, recY = chunks(YKT[h], lambda c: yvnat(h, c), NYKC, ym_sb, "Y")
        oS = w_at.tile([128, QB], DT16, tag="oS", name="oS")
        if qb_i == 0:
            nc.gpsimd.tensor_mul(oS[:, :], pvbS[:, :], recS[:, :])
        else:
            nc.vector.tensor_mul(oS[:, :], pvbS[:, :], recS[:, :])
        oY = w_at.tile([128, QB], DT16, tag="oY", name="oY")
        nc.vector.scalar_tensor_tensor(
            out=oY[:, :], in0=pvbY[:, :], scalar=tg_sb[:, h:h + 1],
            in1=recY[:, :], op0=Alu.mult, op1=Alu.mult)
        if qb_i == 1:
            nc.vector.tensor_add(outT[h][:, q0:q0 + QB], oS[:, :], oY[:, :])
        else:
            nc.gpsimd.tensor_add(outT[h][:, q0:q0 + QB], oS[:, :], oY[:, :])

    def wo_tile(st):
        ob = w_ob.tile([128, D], DT16, tag="obuf", name="obuf")
        for j in range(0, D, JB):
            ps = pp_wo.tile([128, JB], F32, tag="po", name="po")
            for dc in range(HPC):
                nc.tensor.matmul(ps[:, :], outT[dc][:, st * 128:(st + 1) * 128],
                                 wo_sb[:, dc, j:j + JB],
                                 start=(dc == 0), stop=(dc == HPC - 1))
            if (j // JB) % 2 == 0:
                nc.scalar.activation(ob[:, j:j + JB], ps[:, :], AF.Copy)
            else:
                nc.vector.tensor_copy(ob[:, j:j + JB], ps[:, :])
        nc.sync.dma_start(t["out"].ap()[st * 128:(st + 1) * 128, :],
                           ob[:, :])

    for h in range(HPC):
        attend(h, 0)
        q_ln_unit(h, 2, pp_wo)
        q_ln_unit(h, 3, pp_wo)
    wo_tile(0)
    wo_tile(1)
    for h in range(HPC):
        attend(h, 1)
        if h >= 1:
            for st in range(2 * h, 2 * h + 2):
                wo_tile(st)

    # ---- wo tail: wide PSUM tiles, evictions alternating DVE/Act ----
    cm_pv.__exit__(None, None, None)
    cm_sc.__exit__(None, None, None)
    cm_wo2 = tc.tile_pool(name="pp_wo2", bufs=3, space="PSUM")
    pp_wo2 = cm_wo2.__enter__()
    for st in range(8, 16):
        ob = w_ob.tile([128, D], DT16, tag="obuf", name="obuf")
        for half in range(2):
            j0 = half * 1024
            ps = pp_wo2.tile([128, 1024], F32, tag="po2", name="po2")
            for j in range(0, 1024, JB):
                for dc in range(HPC):
                    nc.tensor.matmul(
                        ps[:, j:j + JB],
                        outT[dc][:, st * 128:(st + 1) * 128],
                        wo_sb[:, dc, j0 + j:j0 + j + JB],
                        start=(dc == 0), stop=(dc == HPC - 1))
            if half == 0:
                nc.vector.tensor_copy(ob[:, j0:j0 + 1024], ps[:, :])
            else:
                nc.scalar.activation(ob[:, j0:j0 + 1024], ps[:, :], AF.Copy)
        nc.sync.dma_start(t["out"].ap()[st * 128:(st + 1) * 128, :],
                           ob[:, :])
    cm_wo2.__exit__(None, None, None)

    cm_wops.__exit__(None, None, None)
    cm_wob.__exit__(None, None, None)
    cm_pt.__exit__(None, None, None)
    cm_wat.__exit__(None, None, None)
    cm_wo.__exit__(None, None, None)
    cm_out.__exit__(None, None, None)
    cm_wln2.__exit__(None, None, None)
    cm_wln.__exit__(None, None, None)
    cm_rm.__exit__(None, None, None)
    cm_raw.__exit__(None, None, None)
    cm_consts.__exit__(None, None, None)


def _perm_cols(ncols):
    p = np.arange(ncols).reshape(-1, HD)
    return np.concatenate([p[:, 0::2], p[:, 1::2]], axis=1).reshape(-1)


def _hilo(a, scale):
    """Split a*scale into fp8 hi + lo streams (e4m3, |.| <= 240)."""
    sa = np.clip(a * scale, -240.0, 240.0).astype(np.float32)
    hi = sa.astype(NPFP8)
    lo = (sa - hi.astype(np.float32)).astype(NPFP8)
    return np.ascontiguousarray(hi), np.ascontiguousarray(lo)


def _prep_core_inputs(inputs, core):
    b, g = core // TP, core % TP
    f32 = np.float32
    x = np.asarray(inputs["x"], f32)
    y = np.asarray(inputs["y"], f32)

    qcols = np.arange(g * QW, (g + 1) * QW)
    kcols = np.arange(g * KW, (g + 1) * KW)
    y0 = (4 * g % 8) * HD
    ycols = np.arange(y0, y0 + YW)
    qperm = qcols[_perm_cols(QW)]
    kperm = kcols[_perm_cols(KW)]
    yperm = ycols[_perm_cols(YW)]

    scale = 1.0 / np.sqrt(HD)
    qg = (np.asarray(inputs["q_norm_g"], f32) * scale)[qperm]
    qb = (np.asarray(inputs["q_norm_b"], f32) * scale)[qperm]
    kg = np.asarray(inputs["k_norm_g"], f32)[kperm]
    kb = np.asarray(inputs["k_norm_b"], f32)[kperm]
    kyg = np.asarray(inputs["ky_norm_g"], f32)[yperm]
    kyb = np.asarray(inputs["ky_norm_b"], f32)[yperm]

    cos = np.asarray(inputs["freqs_cos"], f32)[b].T
    sin = np.asarray(inputs["freqs_sin"], f32)[b].T
    CCm = np.concatenate([cos, cos], 0)
    SSm = np.concatenate([-sin, sin], 0)
    swapP = np.zeros((128, 128), f32)
    swapP[np.arange(128), (np.arange(128) + 64) % 128] = 1.0

    xm = np.where(np.asarray(inputs["x_mask"][b]), 0.0, NEG).astype(f32)
    ym = np.where(np.asarray(inputs["y_mask"][b]), 0.0, NEG).astype(f32)
    tgv = np.tanh(np.asarray(inputs["gate"], f32)[4 * g:4 * g + 4])
    tgv = np.broadcast_to(tgv[None, :], (128, YHPC))

    wkv = np.concatenate([np.asarray(inputs["wk"], f32)[:, kperm],
                          np.asarray(inputs["wv"], f32)[:, kcols]], axis=1)
    wy = np.concatenate([np.asarray(inputs["wk_y"], f32)[:, yperm],
                         np.asarray(inputs["wv_y"], f32)[:, ycols]], axis=1)

    def hl(a, scale):
        hi, lo = _hilo(a, scale)
        return np.ascontiguousarray(np.stack([hi, lo], axis=1))

    bf = lambda a: np.ascontiguousarray(a).astype(NP16)
    return {
        "xhl": hl(x[b].T, XS), "yhl": hl(y[b].T, XS),
        "wqhl": hl(np.asarray(inputs["wq"], f32)[:, qperm], WS),
        "wkvhl": hl(wkv, WS), "wyhl": hl(wy, WS),
        "wo": bf(np.asarray(inputs["wo"], f32)[qcols, :]),
        "CC": bf(CCm), "SSp": bf(SSm), "swapP": bf(swapP),
        "qgc": np.ascontiguousarray(qg.reshape(HPC, HD).T).astype(f32),
        "kgc": np.ascontiguousarray(kg.reshape(KVPC, HD).T).astype(f32),
        "kygc": np.ascontiguousarray(kyg.reshape(YHPC, HD).T).astype(f32),
        "qb": np.ascontiguousarray(qb.reshape(HPC, HD).T).astype(f32),
        "kb": np.ascontiguousarray(kb.reshape(KVPC, HD).T).astype(f32),
        "kyb": np.ascontiguousarray(kyb.reshape(YHPC, HD).T).astype(f32),
        "xmask": np.ascontiguousarray(xm.reshape(NKC, 128).T).astype(f32),
        "ymask": np.ascontiguousarray(ym.reshape(NYKC, 128).T).astype(f32),
        "tg": np.ascontiguousarray(tgv).astype(f32),
    }


def _get_runner():
    global _RUNNER
    if _RUNNER is None:
        _RUNNER = _build_program()
    return _RUNNER


def _get_exec():
    """Build (once) a cached jitted shard_map executable for the program."""
    global _EXEC
    if _EXEC is None:
        import jax
        from jax.experimental.shard_map import shard_map
        from jax.sharding import Mesh, NamedSharding, PartitionSpec

        nc = _get_runner()
        from concourse import bass2jax as b2j
        b2j.install_neuronx_cc_hook()

        pname = (nc.partition_id_tensor.name
                 if nc.partition_id_tensor else None)
        in_names, out_names, out_avals = [], [], []
        for alloc in nc.m.functions[0].allocations:
            if not isinstance(alloc, mybir.MemoryLocationSet):
                continue
            name = alloc.memorylocations[0].name
            if alloc.kind == "ExternalInput":
                if name != pname:
                    in_names.append(name)
            elif alloc.kind == "ExternalOutput":
                out_names.append(name)
                out_avals.append(jax.core.ShapedArray(
                    tuple(alloc.tensor_shape), mybir.dt.np(alloc.dtype)))
        n_params = len(in_names)
        all_in = list(in_names + out_names)
        if pname is not None:
            all_in.append(pname)
        all_in = tuple(all_in)
        donate = tuple(range(n_params, n_params + len(out_names)))

        def _body(*args):
            operands = list(args)
            if pname is not None:
                operands.append(b2j.partition_id_tensor())
            outs = b2j._bass_exec_p.bind(
                *operands, out_avals=tuple(out_avals), in_names=all_in,
                out_names=tuple(out_names),
                lowering_input_output_aliases=(),
                sim_require_finite=True, sim_require_nnan=True, nc=nc)
            return tuple(outs)

        devices = jax.devices()[:N_CORES]
        mesh = Mesh(np.asarray(devices), ("core",))
        nin = n_params + len(out_names)
        sharded = jax.jit(
            shard_map(_body, mesh=mesh,
                      in_specs=(PartitionSpec("core"),) * nin,
                      out_specs=(PartitionSpec("core"),) * len(out_names),
                      check_rep=False),
            donate_argnums=donate, keep_unused=True)
        shd = NamedSharding(mesh, PartitionSpec("core"))
        mk0 = [jax.jit(lambda a=a: __import__("jax.numpy", fromlist=["x"]
                                              ).zeros((N_CORES * a.shape[0],)
                                                      + a.shape[1:], a.dtype),
                       out_shardings=shd) for a in out_avals]
        _EXEC = (sharded, in_names, out_names, out_avals, shd, mk0)
    return _EXEC


def _concat_inputs(in_maps):
    sharded, in_names, out_names, out_avals, shd, mk0 = _get_exec()
    return [np.concatenate([np.asarray(in_maps[c][nm])
                            for c in range(N_CORES)], axis=0)
            for nm in in_names]


def _exec(concat_in, device_put=False):
    """Run once; returns {name: full concatenated np array}."""
    import jax
    sharded, in_names, out_names, out_avals, shd, mk0 = _get_exec()
    if device_put:
        concat_in = [jax.device_put(a, shd) for a in concat_in]
    outs = sharded(*concat_in, *[f() for f in mk0])
    return dict(zip(out_names, outs))


def run_on_cores(in_maps, trace=False):
    nc = _get_runner()
    return bass_utils.run_bass_kernel_spmd(
        nc, in_maps, core_ids=list(range(N_CORES)), trace=trace)


def kernel(**inputs):
    in_maps = [_prep_core_inputs(inputs, c) for c in range(N_CORES)]
    outs = _exec(_concat_inputs(in_maps))
    o = np.asarray(outs["out"]).astype(np.float32).reshape(N_CORES, S, D)
    out = np.zeros((B, S, D), np.float32)
    for c in range(N_CORES):
        out[c // TP] += o[c]
    return out


# revision 106
# speedup vs baseline: 1.0034x; 1.0034x over previous
"""Sharded attention kernel for Trainium2 (8 NeuronCores, Bass/Tile).

Module: x->(wq,wk,wv) qk-norm + rope + GQA self-attn  (+)  gated cross-attn
over y->(wk_y,wv_y), then wo.  B=2, S=2048, D=2048, H=16, KV=8, HD=128,
YL=256, YD=1024.

Sharding: 2-way batch DP x 4-way head TP.  Core c handles batch c//4 and
head group g=c%4 (q heads 4g..4g+3, kv heads 2g..2g+1, y-heads
(4g%8)..(4g%8)+3).  wo is row-sharded; the 4 partial outputs per batch are
summed on the host.  The q/k/ky layernorms normalize over the *full* flat
head dim, so each core computes partial (sum, sumsq) stats and three small
in-kernel AllReduces (groups [[0..3],[4..7]]) produce the full-row moments.

Projections run as error-compensated fp8 (e4m3) DoubleRow matmuls: the host
splits x / y / every projection weight into hi+lo fp8 streams (same DMA
bytes as fp16) with fixed power-of-2 scales; on device each contraction
pair-chunk issues three DoubleRow matmuls (hh, hl, lh) into the same fp32
PSUM accumulation, recovering ~2^-8 effective operand precision at 3/4 the
fp16 PE cost.  Evictions fold the descale into the activation-copy scale.
Attention stays fp16: scores are computed transposed (keys on partitions)
so the key mask folds into the exp() bias and P feeds PV untransposed;
softmax skips max-subtraction; denominators accumulate on DVE and reduce
across partitions with a gpsimd partition_all_reduce (Pool engine), keeping
the softmax tail entirely off PE and PSUM.  wo is interleaved into the
second query-block's attention to fill PE while Activation runs exp.
"""
import sys

sys.path.insert(0, "/opt/trn_rl_repo")

import numpy as np
import ml_dtypes

import concourse.bass as bass  # noqa: F401
import concourse.tile as tile
from concourse import bacc, mybir, bass_isa
from concourse import bass_utils
from concourse.masks import make_identity

BF16 = mybir.dt.bfloat16
DT16 = mybir.dt.float16
F32 = mybir.dt.float32
FP8 = mybir.dt.float8e4
NPFP8 = mybir.dt.np(FP8)
NP16 = np.float16
DR = mybir.MatmulPerfMode.DoubleRow

B, S, D, H, KV, YL, YD, HD = 2, 2048, 2048, 16, 8, 256, 1024, 128
N_CORES, TP = 8, 4
HPC, KVPC, YHPC = 4, 2, 4          # q / kv / y heads per core
QW, KW, YW = HPC * HD, KVPC * HD, YHPC * HD   # 512, 256, 512 output cols
NDC, NYC = D // 128, YD // 128     # contraction chunks: 16, 8
NSB, SB = 4, 512                   # seq blocks for projections
NQB, QB = 2, 1024                  # query blocks for attention
JB = 512                           # attention j-chunk (max moving free)
NKC = S // 128                     # 16 key chunks (self)
NYKC = YL // 128                   # 2 key chunks (cross)
NST = S // 128                     # 16 seq tiles for wo
EPS_QK, EPS_KY = 1e-5, 1e-6
NEG = -1.0e30
XS, WS = 32.0, 2048.0              # fixed fp8 scales (power of 2)
DESC = 1.0 / (XS * WS)             # eviction descale

_RUNNER = None
_EXEC = None


def _build_program(use_cc=True):
    nc = bacc.Bacc("TRN2", target_bir_lowering=False, debug=False,
                   num_devices=N_CORES if use_cc else 1)

    def din(name, shape, dt=DT16):
        return nc.dram_tensor(name, shape, dt, kind="ExternalInput")

    t = dict(
        xhl=din("xhl", [D, 2, S], FP8),
        yhl=din("yhl", [YD, 2, YL], FP8),
        wqhl=din("wqhl", [D, 2, QW], FP8),
        wkvhl=din("wkvhl", [D, 2, 2 * KW], FP8),
        wyhl=din("wyhl", [YD, 2, 2 * YW], FP8),
        wo=din("wo", [QW, D]),
        CC=din("CC", [128, S]),
        SSp=din("SSp", [128, S]),
        swapP=din("swapP", [128, 128]),
        qgc=din("qgc", [128, HPC], F32),
        kgc=din("kgc", [128, KVPC], F32),
        kygc=din("kygc", [128, YHPC], F32),
        qb=din("qb", [128, HPC], F32),
        kb=din("kb", [128, KVPC], F32),
        kyb=din("kyb", [128, YHPC], F32),
        xmask=din("xmask", [128, NKC], F32),
        ymask=din("ymask", [128, NYKC], F32),
        tg=din("tg", [128, YHPC], F32),
        out=nc.dram_tensor("out", [S, D], DT16, kind="ExternalOutput"),
        kin=nc.dram_tensor("kin", [2, S], F32),
        kout=nc.dram_tensor("kout", [2, S], F32),
        kyin=nc.dram_tensor("kyin", [2, YL], F32),
        kyout=nc.dram_tensor("kyout", [2, YL], F32),
        qin=nc.dram_tensor("qin", [2, S], F32),
        qout=nc.dram_tensor("qout", [2, S], F32),
        lnr=nc.dram_tensor("lnr", [6, S], DT16),
        groups=[[0, 1, 2, 3], [4, 5, 6, 7]],
        use_cc=use_cc,
    )

    with tile.TileContext(nc) as tc:
        _emit(nc, tc, t)
    nc.compile()
    return nc


def _allreduce(nc, t, name, nh, h):
    """AllReduce of one half-major block of the partial LN stats (cc) /
    local copy (no-cc).  The stats tensors are laid out half-major:
    block h holds [sum_cols | sq_cols] for its column range contiguously,
    so the collective input is a single contiguous run."""
    Alu = mybir.AluOpType
    tin, tout = t[name + "in"], t[name + "out"]
    n = 2 * tin.shape[1] // nh
    src = bass.AP(tensor=tin.ap().tensor, offset=h * n, ap=[[1, n]])
    dst = bass.AP(tensor=tout.ap().tensor, offset=h * n, ap=[[1, n]])
    if t["use_cc"]:
        nc.gpsimd.collective_compute(
            "AllReduce", Alu.add, replica_groups=t["groups"],
            ins=[src], outs=[dst])


def _st(t, name):
    """Stats source for moments: the AllReduce output when collectives
    run; the local partials directly in the single-core timing variant
    (the collective adds no local engine work there)."""
    return t[name + "out"] if t["use_cc"] else t[name + "in"]


def _emit(nc, tc, t):
    AF = mybir.ActivationFunctionType
    Alu = mybir.AluOpType

    cm_consts = tc.tile_pool(name="consts", bufs=1)
    consts = cm_consts.__enter__()

    # ---------------- constants / small inputs ----------------
    ident = consts.tile([128, 128], DT16, tag="ident", name="ident")
    make_identity(nc, ident[:, :])
    ones_col = consts.tile([128, 1], DT16, tag="ones_col", name="ones_col")
    nc.vector.memset(ones_col[:, :], 1.0)
    ones_bf = consts.tile([128, 1], BF16, tag="ones_bf", name="ones_bf")
    nc.vector.memset(ones_bf[:, :], 1.0)
    swp = consts.tile([128, 128], DT16, tag="swp", name="swp")
    cc = consts.tile([128, S], DT16, tag="cc", name="cc")
    ssp = consts.tile([128, S], DT16, tag="ssp", name="ssp")
    qg_sb = consts.tile([128, HPC], F32, tag="qgc", name="qgc")
    nc.gpsimd.dma_start(qg_sb[:, :], t["qgc"].ap())
    kg_sb = consts.tile([128, KVPC], F32, tag="kgc", name="kgc")
    nc.gpsimd.dma_start(kg_sb[:, :], t["kgc"].ap())
    kyg_sb = consts.tile([128, YHPC], F32, tag="kygc", name="kygc")
    nc.gpsimd.dma_start(kyg_sb[:, :], t["kygc"].ap())
    qb_sb = consts.tile([128, HPC], F32, tag="qb", name="qb")
    nc.gpsimd.dma_start(qb_sb[:, :], t["qb"].ap())
    kb_sb = consts.tile([128, KVPC], F32, tag="kb", name="kb")
    nc.gpsimd.dma_start(kb_sb[:, :], t["kb"].ap())
    kyb_sb = consts.tile([128, YHPC], F32, tag="kyb", name="kyb")
    nc.gpsimd.dma_start(kyb_sb[:, :], t["kyb"].ap())
    xm_sb = consts.tile([128, NKC], F32, tag="xm", name="xm")
    nc.gpsimd.dma_start(xm_sb[:, :], t["xmask"].ap())
    ym_sb = consts.tile([128, NYKC], F32, tag="ym", name="ym")
    nc.gpsimd.dma_start(ym_sb[:, :], t["ymask"].ap())
    tg_sb = consts.tile([128, YHPC], F32, tag="tg", name="tg")
    nc.gpsimd.dma_start(tg_sb[:, :], t["tg"].ap())

    # ---------------- phase-1 pools ----------------
    cm_raw = tc.tile_pool(name="p_raw", bufs=1)
    p_raw = cm_raw.__enter__()
    cm_w = tc.tile_pool(name="p_w", bufs=1)
    p_w = cm_w.__enter__()
    cm_x = tc.tile_pool(name="p_x", bufs=1)
    p_x = cm_x.__enter__()
    cm_wsq = tc.tile_pool(name="w_sq", bufs=2)
    w_sq = cm_wsq.__enter__()
    cm_stg = tc.tile_pool(name="w_stg", bufs=2)
    w_stg = cm_stg.__enter__()
    cm_ar = tc.tile_pool(name="w_ar", bufs=2)
    w_ar = cm_ar.__enter__()

    def stat_out(tname, nh, stg, col0, blk):
        """Write the accumulated (sum | sumsq) staging row with one DMA
        into the half-major stats layout."""
        hw_ = t[tname].shape[1] // nh          # cols per half
        h, rel = col0 // hw_, col0 % hw_
        dst = bass.AP(tensor=t[tname].ap().tensor, offset=2 * h * hw_ + rel,
                      ap=[[hw_, 2], [1, blk]])
        nc.scalar.dma_start(dst, stg[:, :2 * blk])

    # hi/lo-packed fp8 tiles: dim -2 selects the stream (0=hi, 1=lo)
    wq_sb = p_w.tile([128, NDC, 2, QW], FP8, tag="wq", name="wq")
    wkv_sb = p_w.tile([128, NDC, 2, 2 * KW], FP8, tag="wkv", name="wkv")
    wy_sb = p_w.tile([128, NYC, 2, 2 * YW], FP8, tag="wy", name="wy")
    y_sb = p_w.tile([128, NYC, 2, YL], FP8, tag="y", name="y")

    xr = t["xhl"].ap().rearrange("(c p) two s -> p c two s", p=128)
    wq_r = t["wqhl"].ap().rearrange("(c p) two m -> p c two m", p=128)
    wkv_r = t["wkvhl"].ap().rearrange("(c p) two m -> p c two m", p=128)
    wy_r = t["wyhl"].ap().rearrange("(c p) two m -> p c two m", p=128)
    y_r = t["yhl"].ap().rearrange("(c p) two s -> p c two s", p=128)

    # x tiles for all 4 seq blocks stay resident (both passes read them)
    xts = [p_x.tile([128, NDC, 2, SB], FP8, tag=f"x_{sb}", name=f"x_{sb}")
           for sb in range(NSB)]

    # load order: kv pass runs first, so wkv-hi + x0-hi strips lead (the
    # hh-term sweeps start on them), then the lo strips, then the rest.
    for s in (0, 1):
        for c in range(0, NDC, 4):
            nc.sync.dma_start(wkv_sb[:, c:c + 4, s, :],
                              wkv_r[:, c:c + 4, s, :])
            nc.sync.dma_start(xts[0][:, c:c + 4, s, :],
                              xr[:, c:c + 4, s, 0:SB])
    for sb in range(1, NSB):
        for c in range(0, NDC, 8):
            for s in (0, 1):
                nc.sync.dma_start(xts[sb][:, c:c + 8, s, :],
                                  xr[:, c:c + 8, s, sb * SB:(sb + 1) * SB])
    for c in range(0, NDC, 4):
        for s in (0, 1):
            nc.sync.dma_start(wq_sb[:, c:c + 4, s, :],
                              wq_r[:, c:c + 4, s, :])
    nc.sync.dma_start(swp[:, :], t["swapP"].ap())
    nc.sync.dma_start(cc[:, :], t["CC"].ap())
    nc.sync.dma_start(ssp[:, :], t["SSp"].ap())
    for s in (0, 1):
        nc.sync.dma_start(y_sb[:, :, s, :], y_r[:, :, s, :])
    for s in (0, 1):
        nc.sync.dma_start(wy_sb[:, :, s, :], wy_r[:, :, s, :])

    # raw projection outputs (fp16); later reused in place for QT/KT/vnat
    qraw = [p_raw.tile([128, S], DT16, tag=f"qraw{i}", name=f"qraw{i}")
            for i in range(HPC)]
    kraw = [p_raw.tile([128, S], DT16, tag=f"kraw{i}", name=f"kraw{i}")
            for i in range(KVPC)]
    vraw = [p_raw.tile([128, S], DT16, tag=f"vraw{i}", name=f"vraw{i}")
            for i in range(KVPC)]
    ykraw = [p_raw.tile([128, YL], DT16, tag=f"ykraw{i}", name=f"ykraw{i}")
             for i in range(YHPC)]
    yvraw = [p_raw.tile([128, YL], DT16, tag=f"yvraw{i}", name=f"yvraw{i}")
             for i in range(YHPC)]

    cm_psA = tc.tile_pool(name="pp_projA", bufs=2, space="PSUM")
    pp_proj = cm_psA.__enter__()

    def proj_fp8(w_t, x_t, npair, col0, blk, ps):
        """3-term compensated fp8 DoubleRow accumulation into ps.
        hh terms sweep first so compute can start before lo streams land."""
        first = True
        for (ws_, xs_) in ((0, 0), (0, 1), (1, 0)):
            for c in range(npair):
                nc.tensor.matmul(
                    ps[:, :blk],
                    w_t[:, 2 * c:2 * c + 2, ws_, col0:col0 + 128],
                    x_t[:, 2 * c:2 * c + 2, xs_, :blk],
                    start=first,
                    stop=((ws_, xs_) == (1, 0) and c == npair - 1),
                    perf_mode=DR)
                first = False

    def proj_block(w_t, x_t, npair, col0, dst, sb, blk,
                   stg=None, first=False):
        ps = pp_proj.tile([128, SB], F32, tag="proj", name="proj")
        proj_fp8(w_t, x_t, npair, col0, blk, ps)
        nc.scalar.activation(dst[:, sb * blk:(sb + 1) * blk], ps[:, :blk],
                             AF.Copy, scale=DESC)
        if stg is not None:
            # LN stats off PE: Pool partition-reduces the evicted tile and
            # its square; DVE accumulates the row into the staging tile
            sq = w_sq.tile([128, SB], BF16, tag="sqscratch", name="sqscratch")
            nc.scalar.activation(sq[:, :blk], ps[:, :blk], AF.Square,
                                 scale=DESC)
            ars = w_ar.tile([128, SB], DT16, tag="ars", name="ars")
            nc.gpsimd.partition_all_reduce(
                ars[:, :blk], dst[:, sb * blk:(sb + 1) * blk],
                channels=128, reduce_op=bass_isa.ReduceOp.add)
            arq = w_ar.tile([128, SB], BF16, tag="arq", name="arq")
            nc.gpsimd.partition_all_reduce(
                arq[:, :blk], sq[:, :blk],
                channels=128, reduce_op=bass_isa.ReduceOp.add)
            if first:
                nc.vector.tensor_copy(stg[0:1, :blk], ars[0:1, :blk])
                nc.vector.tensor_copy(stg[0:1, blk:2 * blk], arq[0:1, :blk])
            else:
                nc.vector.tensor_add(stg[0:1, :blk], stg[0:1, :blk],
                                     ars[0:1, :blk])
                nc.vector.tensor_add(stg[0:1, blk:2 * blk],
                                     stg[0:1, blk:2 * blk], arq[0:1, :blk])

    cm_rm = tc.tile_pool(name="rows_m", bufs=2, side="right")
    rows_m = cm_rm.__enter__()
    cm_wln = tc.tile_pool(name="w_ln", bufs=1, side="right")
    w_ln = cm_wln.__enter__()
    cm_wln2 = tc.tile_pool(name="w_ln2", bufs=2, side="right")
    w_ln2 = cm_wln2.__enter__()

    def moments(src_t, nh, n, inv_scale, eps, length, r_rstd, col0=0,
                ncols=None):
        """src_t is half-major (sum cols | sq cols per half); process the
        half starting at column col0.  Partition-parallel math on
        [128, ncols/128]; rstd and -mu*rstd slices land in lnr rows
        (r_rstd, r_rstd+1) via one DMA each way."""
        ncols = ncols or length
        J = ncols // 128
        hw_ = length // nh

        ab = rows_m.tile([128, 2, 16], F32, tag="mab", name="mab")
        src = bass.AP(tensor=src_t.ap().tensor,
                      offset=2 * (col0 // hw_) * hw_ + col0 % hw_,
                      ap=[[J, 128], [hw_, 2], [1, J]])
        nc.scalar.dma_start(ab[:, :, :J], src)
        a, b = ab[:, 0, :J], ab[:, 1, :J]
        nc.vector.tensor_scalar_mul(a, a, inv_scale / n)
        nc.vector.tensor_scalar_mul(b, b, inv_scale / n)
        c_ = rows_m.tile([128, 16], F32, tag="mc", name="mc")
        nc.vector.tensor_mul(c_[:, :J], a, a)
        nc.vector.tensor_tensor(b, b, c_[:, :J], Alu.subtract)
        nc.vector.tensor_scalar_add(b, b, eps)
        # rsqrt via exp(-0.5*ln(var)): stays in the exp activation table
        # (no table switch before attention); the Newton step below refines.
        nc.scalar.activation(c_[:, :J], b, AF.Ln)
        nc.scalar.activation(c_[:, :J], c_[:, :J], AF.Exp, scale=-0.5)
        d = rows_m.tile([128, 16], F32, tag="md", name="md")
        nc.vector.tensor_mul(d[:, :J], c_[:, :J], c_[:, :J])
        nc.vector.tensor_mul(d[:, :J], d[:, :J], b)
        nc.vector.tensor_scalar(out=d[:, :J], in0=d[:, :J],
                                scalar1=-0.5, scalar2=1.5,
                                op0=Alu.mult, op1=Alu.add)
        nc.vector.tensor_mul(c_[:, :J], c_[:, :J], d[:, :J])
        nc.vector.tensor_mul(a, a, c_[:, :J])
        nc.vector.tensor_scalar_mul(a, a, -1.0)
        ra = rows_m.tile([128, 2, 16], DT16, tag="mra", name="mra")
        nc.vector.tensor_copy(ra[:, 0, :J], c_[:, :J])
        nc.vector.tensor_copy(ra[:, 1, :J], a)
        out_r = bass.AP(tensor=t["lnr"].ap().tensor,
                        offset=r_rstd * S + col0,
                        ap=[[J, 128], [S, 2], [1, J]])
        nc.sync.dma_start(out_r, ra[:, :, :J])

    def dma_bcast(rgng, r_rstd, col0, ncols):
        """One DMA: broadcast lnr rows (r_rstd, r_rstd+1) column slice into
        the [128, 2, *] rgng tile."""
        src_ap = bass.AP(tensor=t["lnr"].ap().tensor,
                         offset=r_rstd * S + col0,
                         ap=[[0, 128], [S, 2], [1, ncols]])
        nc.sync.dma_start(rgng[:, :, col0:col0 + ncols], src_ap)

    cm_swp = tc.tile_pool(name="pp_swap", bufs=2, space="PSUM")
    pp_swap = cm_swp.__enter__()

    # broadcast-row tiles for the three LN streams ([:, 0, :]=rstd*,
    # [:, 1, :]=-mu*rstd)
    q_rr = w_ln.tile([128, 2, S], DT16, tag="q_rr", name="q_rr")
    k_rr = w_ln.tile([128, 2, S], DT16, tag="k_rr", name="k_rr")
    ky_rr = w_ln.tile([128, 2, YL], DT16, tag="ky_rr", name="ky_rr")

    def ln_unit(raw, rr, g_col, b_col, col0, ncols, rope, ps_pool=None,
                eng="dve"):
        """LayerNorm (+optional rope) of one head's column slice, in place.
        In phase 1 the per-head gain/bias and swap eviction run on
        Activation (per-partition scalars) to unload DVE at the boundary;
        units deferred into the attention region (ps_pool set) keep those
        ops on DVE so they don't compete with exp."""
        sl = slice(col0, col0 + ncols)
        in_attn = ps_pool is not None
        t1 = w_ln2.tile([128, SB], DT16, tag="lnt1", name="lnt1")
        nc.vector.tensor_mul(t1[:, :ncols], raw[:, sl], rr[:, 0, sl])
        nc.vector.tensor_add(t1[:, :ncols], t1[:, :ncols], rr[:, 1, sl])
        if not rope:
            if eng == "act":
                nc.scalar.activation(raw[:, sl], t1[:, :ncols], AF.Identity,
                                     bias=b_col, scale=g_col)
            else:
                nc.vector.tensor_scalar(out=raw[:, sl], in0=t1[:, :ncols],
                                        scalar1=g_col, scalar2=b_col,
                                        op0=Alu.mult, op1=Alu.add)
            return
        if eng == "act":
            nc.scalar.activation(t1[:, :ncols], t1[:, :ncols], AF.Identity,
                                 bias=b_col, scale=g_col)
        else:
            nc.vector.tensor_scalar(out=t1[:, :ncols], in0=t1[:, :ncols],
                                    scalar1=g_col, scalar2=b_col,
                                    op0=Alu.mult, op1=Alu.add)
        sw = w_ln2.tile([128, SB], DT16, tag="swap", name="swap")
        pool = ps_pool or pp_swap
        ps = pool.tile([128, JB], F32, tag="tp" if ps_pool is None else "po",
                       name="lnswp")
        nc.tensor.matmul(ps[:, :ncols], swp[:, :], t1[:, :ncols],
                         start=True, stop=True)
        if eng == "act":
            nc.scalar.activation(sw[:, :ncols], ps[:, :ncols], AF.Copy)
        else:
            nc.vector.tensor_copy(sw[:, :ncols], ps[:, :ncols])
        nc.vector.tensor_mul(t1[:, :ncols], t1[:, :ncols], cc[:, sl])
        nc.vector.tensor_mul(sw[:, :ncols], sw[:, :ncols], ssp[:, sl])
        nc.vector.tensor_add(raw[:, sl], t1[:, :ncols], sw[:, :ncols])

    def vtrans(raw, c):
        tp = pp_swap.tile([128, 128], DT16, tag="vtp", name="vtp")
        nc.tensor.transpose(tp[:, :], raw[:, c * 128:(c + 1) * 128],
                            ident[:, :])
        nc.scalar.activation(raw[:, c * 128:(c + 1) * 128], tp[:, :],
                             AF.Copy)

    def q_ln_unit(h, sb, ps_pool=None, eng="dve"):
        ln_unit(qraw[h], q_rr, qg_sb[:, h:h + 1],
                qb_sb[:, h:h + 1], sb * SB, SB, True, ps_pool, eng)

    def k_ln_sb(sb):
        for i in range(KVPC):
            ln_unit(kraw[i], k_rr, kg_sb[:, i:i + 1],
                    kb_sb[:, i:i + 1], sb * SB, SB, True)

    def proj_pair(w_t, x_t, npair, col0s, dsts, sb, blk, stg, first0):
        """Two colchunks term-major: all hh sweeps first so compute can
        start before the lo fp8 streams arrive (first DMA-paced block)."""
        pss = [pp_proj.tile([128, SB], F32, tag="proj", name="proj")
               for _ in col0s]
        for ti, (ws_, xs_) in enumerate(((0, 0), (0, 1), (1, 0))):
            for j, col0 in enumerate(col0s):
                for c in range(npair):
                    nc.tensor.matmul(
                        pss[j][:, :blk],
                        w_t[:, 2 * c:2 * c + 2, ws_, col0:col0 + 128],
                        x_t[:, 2 * c:2 * c + 2, xs_, :blk],
                        start=(ti == 0 and c == 0),
                        stop=(ti == 2 and c == npair - 1),
                        perf_mode=DR)
        for j, dst in enumerate(dsts):
            nc.scalar.activation(dst[:, sb * blk:(sb + 1) * blk],
                                 pss[j][:, :blk], AF.Copy, scale=DESC)
            if stg is not None:
                sq = w_sq.tile([128, SB], BF16, tag="sqscratch",
                               name="sqscratch")
                nc.scalar.activation(sq[:, :blk], pss[j][:, :blk], AF.Square,
                                     scale=DESC)
                ars = w_ar.tile([128, SB], DT16, tag="ars", name="ars")
                nc.gpsimd.partition_all_reduce(
                    ars[:, :blk], dst[:, sb * blk:(sb + 1) * blk],
                    channels=128, reduce_op=bass_isa.ReduceOp.add)
                arq = w_ar.tile([128, SB], BF16, tag="arq", name="arq")
                nc.gpsimd.partition_all_reduce(
                    arq[:, :blk], sq[:, :blk],
                    channels=128, reduce_op=bass_isa.ReduceOp.add)
                if first0 and j == 0:
                    nc.vector.tensor_copy(stg[0:1, :blk], ars[0:1, :blk])
                    nc.vector.tensor_copy(stg[0:1, blk:2 * blk],
                                          arq[0:1, :blk])
                else:
                    nc.vector.tensor_add(stg[0:1, :blk], stg[0:1, :blk],
                                         ars[0:1, :blk])
                    nc.vector.tensor_add(stg[0:1, blk:2 * blk],
                                         stg[0:1, blk:2 * blk],
                                         arq[0:1, :blk])

    # ============ phase 1a: k/v projections + stats; AR in halves ==========
    for sb in range(NSB):
        xt = xts[sb]
        kstg = w_stg.tile([1, 2 * SB], F32, tag="stg", name="stg")
        if sb == 0:
            proj_pair(wkv_sb, xt, NDC // 2, [0, 128], kraw, sb, SB,
                      kstg, True)
            proj_pair(wkv_sb, xt, NDC // 2, [KW, KW + 128], vraw, sb, SB,
                      None, False)
        else:
            for i in range(KVPC):
                proj_block(wkv_sb, xt, NDC // 2, i * 128, kraw[i], sb, SB,
                           kstg, first=(i == 0))
            for i in range(KVPC):
                proj_block(wkv_sb, xt, NDC // 2, KW + i * 128, vraw[i],
                           sb, SB)
        stat_out("kin", 2, kstg, sb * SB, SB)
        if sb == 1:
            _allreduce(nc, t, "k", 2, 0)
        if sb == 2:
            moments(_st(t, "k"), 2, KV * HD, 1.0, EPS_QK, S, 2, 0, 2 * SB)
            dma_bcast(k_rr, 2, 0, 2 * SB)
        for i in range(KVPC):
            for c in range(4 * sb, 4 * sb + 4):
                vtrans(vraw[i], c)
        if sb == 3:
            k_ln_sb(0)
    k_ln_sb(1)
    _allreduce(nc, t, "k", 2, 1)
    moments(_st(t, "k"), 2, KV * HD, 1.0, EPS_QK, S, 2, 2 * SB, 2 * SB)
    dma_bcast(k_rr, 2, 2 * SB, 2 * SB)

    # ============ phase 1b: q projections; k-LN tail interleaved; the q
    # stats AR runs in halves so query-block-0's LN lands inside the pass ===
    for sb in range(NSB):
        xt = xts[sb]
        qstg = w_stg.tile([1, 2 * SB], F32, tag="stg", name="stg")
        for i in range(HPC):
            proj_block(wq_sb, xt, NDC // 2, i * 128, qraw[i], sb, SB,
                       qstg, first=(i == 0))
        stat_out("qin", 2, qstg, sb * SB, SB)
        if sb == 0:
            k_ln_sb(2)
        if sb == 1:
            _allreduce(nc, t, "q", 2, 0)
            k_ln_sb(3)
        if sb == 2:
            moments(_st(t, "q"), 2, H * HD, 1.0, EPS_QK, S, 0, 0, 2 * SB)
            dma_bcast(q_rr, 0, 0, 2 * SB)
        if sb == 3:
            for h in range(HPC):
                q_ln_unit(h, 0)
    for h in range(HPC):
        q_ln_unit(h, 1, eng=("act" if h % 2 == 0 else "dve"))
    _allreduce(nc, t, "q", 2, 1)
    moments(_st(t, "q"), 2, H * HD, 1.0, EPS_QK, S, 0, 2 * SB, 2 * SB)
    dma_bcast(q_rr, 0, 2 * SB, 2 * SB)

    # ---- y projections ----
    ystg = w_stg.tile([1, 2 * SB], F32, tag="stg", name="stg")
    for i in range(YHPC):
        proj_block(wy_sb, y_sb, NYC // 2, i * 128, ykraw[i], 0, YL,
                   ystg, first=(i == 0))
    for i in range(YHPC):
        proj_block(wy_sb, y_sb, NYC // 2, YW + i * 128, yvraw[i], 0, YL)
    stat_out("kyin", 1, ystg, 0, YL)

    _allreduce(nc, t, "ky", 1, 0)

    for i in range(YHPC):
        for c in range(NYKC):
            vtrans(yvraw[i], c)

    # ---- ky LN (no rope, no PE work) ----
    moments(_st(t, "ky"), 1, KV * HD, 0.5, EPS_KY, YL, 4)
    dma_bcast(ky_rr, 4, 0, YL)
    for i in range(YHPC):
        ln_unit(ykraw[i], ky_rr, kyg_sb[:, i:i + 1],
                kyb_sb[:, i:i + 1], 0, YL, False, eng="act")

    # q-LN for the query-block-1 slices is deferred into the qb0 attention
    # region (emitted inside the attention loop below)

    QT, KT, YKT = qraw, kraw, ykraw

    def vnat(i, c):
        return vraw[i][:, c * 128:(c + 1) * 128]

    def yvnat(i, c):
        return yvraw[i][:, c * 128:(c + 1) * 128]

    cm_swp.__exit__(None, None, None)
    cm_psA.__exit__(None, None, None)
    cm_ar.__exit__(None, None, None)
    cm_stg.__exit__(None, None, None)
    cm_wsq.__exit__(None, None, None)
    cm_x.__exit__(None, None, None)
    cm_w.__exit__(None, None, None)

    # ============ attention + wo ============
    cm_out = tc.tile_pool(name="p_out", bufs=1)
    p_out = cm_out.__enter__()
    outT = [p_out.tile([128, S], DT16, tag=f"outT{h}", name=f"outT{h}")
            for h in range(HPC)]
    cm_wo = tc.tile_pool(name="p_wo", bufs=1)
    p_wo = cm_wo.__enter__()
    wo_sb = p_wo.tile([128, HPC, D], DT16, tag="wo", name="wo")
    nc.sync.dma_start(wo_sb[:, :, :],
                      t["wo"].ap().rearrange("(c p) m -> p c m", p=128))
    cm_wat = tc.tile_pool(name="w_at", bufs=3)
    w_at = cm_wat.__enter__()
    cm_pt = tc.tile_pool(name="w_pt", bufs=7)
    w_pt = cm_pt.__enter__()
    cm_wob = tc.tile_pool(name="w_ob", bufs=2)
    w_ob = cm_wob.__enter__()

    cm_wops = tc.tile_pool(name="pp_wo", bufs=2, space="PSUM")
    cm_sc = tc.tile_pool(name="pp_sc", bufs=2, space="PSUM")
    cm_pv = tc.tile_pool(name="pp_pv", bufs=1, space="PSUM")
    pp_wo = cm_wops.__enter__()
    pp_sc = cm_sc.__enter__()
    pp_pv = cm_pv.__enter__()

    def attend(h, qb_i):
        """Self + gated cross attention for query block qb_i of head h."""
        q0 = qb_i * QB
        pv = pp_pv.tile([128, QB], F32, tag="pv", name="pv")

        def chunks(KT_h, vn, nkc, mask_sb, acc_tag):
            acc = w_at.tile([128, QB], DT16, tag=acc_tag, name=acc_tag)
            accB = (w_at.tile([128, QB], DT16, tag=acc_tag + "B",
                              name=acc_tag + "B") if nkc > 4 else None)
            ptA0 = ptB0 = None

            def emit_sc(c):
                sc = pp_sc.tile([128, QB], F32, tag="sc", name="sc")
                for j in range(0, QB, JB):
                    nc.tensor.matmul(sc[:, j:j + JB],
                                     KT_h[:, c * 128:(c + 1) * 128],
                                     QT[h][:, q0 + j:q0 + j + JB],
                                     start=True, stop=True)
                return sc

            # software pipeline: scores one chunk ahead so PE never
            # blocks behind exp(c) when issuing pv(c)
            sc_cur = emit_sc(0)
            for c in range(nkc):
                pt = w_pt.tile([128, QB], DT16, tag="ptile", name="ptile")
                nc.scalar.activation(pt[:, :], sc_cur[:, :], AF.Exp,
                                     bias=mask_sb[:, c:c + 1])
                if c + 1 < nkc:
                    sc_cur = emit_sc(c + 1)
                for j in range(0, QB, JB):
                    nc.tensor.matmul(pv[:, j:j + JB], vn(c),
                                     pt[:, j:j + JB],
                                     start=(c == 0), stop=(c == nkc - 1))
                # two parallel accumulation chains; each chain's first two
                # tiles fuse into one add (no initial copy)
                if accB is not None and c % 4 == 3 and c < nkc - 4:
                    if c == 3:
                        ptB0 = pt
                    elif ptB0 is not None:
                        nc.gpsimd.tensor_add(accB[:, :], ptB0[:, :], pt[:, :])
                        ptB0 = None
                    else:
                        nc.gpsimd.tensor_add(accB[:, :], accB[:, :],
                                             pt[:, :])
                elif c == 0:
                    ptA0 = pt
                elif ptA0 is not None:
                    nc.vector.tensor_add(acc[:, :], ptA0[:, :], pt[:, :])
                    ptA0 = None
                else:
                    nc.vector.tensor_add(acc[:, :], acc[:, :], pt[:, :])
            if accB is not None:
                nc.vector.tensor_add(acc[:, :], acc[:, :], accB[:, :])
            pvb = w_at.tile([128, QB], DT16, tag="pvb" + acc_tag,
                            name="pvb" + acc_tag)
            nc.vector.tensor_copy(pvb[:, :], pv[:, :])
            ar = w_at.tile([128, QB], DT16, tag="ar" + acc_tag,
                           name="ar" + acc_tag)
            nc.gpsimd.partition_all_reduce(ar[:, :], acc[:, :], channels=128,
                                           reduce_op=bass_isa.ReduceOp.add)
            with nc.allow_low_precision(reason="fp16 softmax denominators"):
                nc.vector.reciprocal(ar[:, :], ar[:, :])
            return pvb, ar

        pvbS, recS = chunks(KT[h // 2], lambda c: vnat(h // 2, c), NKC,
                            xm_sb, "S")
        pvbY, recY = chunks(YKT[h], lambda c: yvnat(h, c), NYKC, ym_sb, "Y")
        oS = w_at.tile([128, QB], DT16, tag="oS", name="oS")
        if qb_i == 0:
            nc.gpsimd.tensor_mul(oS[:, :], pvbS[:, :], recS[:, :])
        else:
            nc.vector.tensor_mul(oS[:, :], pvbS[:, :], recS[:, :])
        oY = w_at.tile([128, QB], DT16, tag="oY", name="oY")
        nc.vector.scalar_tensor_tensor(
            out=oY[:, :], in0=pvbY[:, :], scalar=tg_sb[:, h:h + 1],
            in1=recY[:, :], op0=Alu.mult, op1=Alu.mult)
        if qb_i == 1:
            nc.vector.tensor_add(outT[h][:, q0:q0 + QB], oS[:, :], oY[:, :])
        else:
            nc.gpsimd.tensor_add(outT[h][:, q0:q0 + QB], oS[:, :], oY[:, :])

    def wo_tile(st):
        ob = w_ob.tile([128, D], DT16, tag="obuf", name="obuf")
        for j in range(0, D, JB):
            ps = pp_wo.tile([128, JB], F32, tag="po", name="po")
            for dc in range(HPC):
                nc.tensor.matmul(ps[:, :], outT[dc][:, st * 128:(st + 1) * 128],
                                 wo_sb[:, dc, j:j + JB],
                                 start=(dc == 0), stop=(dc == HPC - 1))
            if (j // JB) % 2 == 0:
                nc.scalar.activation(ob[:, j:j + JB], ps[:, :], AF.Copy)
            else:
                nc.vector.tensor_copy(ob[:, j:j + JB], ps[:, :])
        nc.sync.dma_start(t["out"].ap()[st * 128:(st + 1) * 128, :],
                           ob[:, :])

    for h in range(HPC):
        attend(h, 0)
        q_ln_unit(h, 2, pp_wo)
        q_ln_unit(h, 3, pp_wo)
    wo_tile(0)
    wo_tile(1)
    for h in range(HPC):
        attend(h, 1)
        if h >= 1:
            for st in range(2 * h, 2 * h + 2):
                wo_tile(st)

    # ---- wo tail: wide PSUM tiles, evictions alternating DVE/Act ----
    cm_pv.__exit__(None, None, None)
    cm_sc.__exit__(None, None, None)
    cm_wo2 = tc.tile_pool(name="pp_wo2", bufs=3, space="PSUM")
    pp_wo2 = cm_wo2.__enter__()
    for st in range(8, 16):
        ob = w_ob.tile([128, D], DT16, tag="obuf", name="obuf")
        for half in range(2):
            j0 = half * 1024
            ps = pp_wo2.tile([128, 1024], F32, tag="po2", name="po2")
            for j in range(0, 1024, JB):
                for dc in range(HPC):
                    nc.tensor.matmul(
                        ps[:, j:j + JB],
                        outT[dc][:, st * 128:(st + 1) * 128],
                        wo_sb[:, dc, j0 + j:j0 + j + JB],
                        start=(dc == 0), stop=(dc == HPC - 1))
            if half == 0:
                nc.vector.tensor_copy(ob[:, j0:j0 + 1024], ps[:, :])
            else:
                nc.scalar.activation(ob[:, j0:j0 + 1024], ps[:, :], AF.Copy)
        nc.sync.dma_start(t["out"].ap()[st * 128:(st + 1) * 128, :],
                           ob[:, :])
    cm_wo2.__exit__(None, None, None)

    cm_wops.__exit__(None, None, None)
    cm_wob.__exit__(None, None, None)
    cm_pt.__exit__(None, None, None)
    cm_wat.__exit__(None, None, None)
    cm_wo.__exit__(None, None, None)
    cm_out.__exit__(None, None, None)
    cm_wln2.__exit__(None, None, None)
    cm_wln.__exit__(None, None, None)
    cm_rm.__exit__(None, None, None)
    cm_raw.__exit__(None, None, None)
    cm_consts.__exit__(None, None, None)


def _perm_cols(ncols):
    p = np.arange(ncols).reshape(-1, HD)
    return np.concatenate([p[:, 0::2], p[:, 1::2]], axis=1).reshape(-1)


def _hilo(a, scale):
    """Split a*scale into fp8 hi + lo streams (e4m3, |.| <= 240)."""
    sa = np.clip(a * scale, -240.0, 240.0).astype(np.float32)
    hi = sa.astype(NPFP8)
    lo = (sa - hi.astype(np.float32)).astype(NPFP8)
    return np.ascontiguousarray(hi), np.ascontiguousarray(lo)


def _prep_core_inputs(inputs, core):
    b, g = core // TP, core % TP
    f32 = np.float32
    x = np.asarray(inputs["x"], f32)
    y = np.asarray(inputs["y"], f32)

    qcols = np.arange(g * QW, (g + 1) * QW)
    kcols = np.arange(g * KW, (g + 1) * KW)
    y0 = (4 * g % 8) * HD
    ycols = np.arange(y0, y0 + YW)
    qperm = qcols[_perm_cols(QW)]
    kperm = kcols[_perm_cols(KW)]
    yperm = ycols[_perm_cols(YW)]

    scale = 1.0 / np.sqrt(HD)
    qg = (np.asarray(inputs["q_norm_g"], f32) * scale)[qperm]
    qb = (np.asarray(inputs["q_norm_b"], f32) * scale)[qperm]
    kg = np.asarray(inputs["k_norm_g"], f32)[kperm]
    kb = np.asarray(inputs["k_norm_b"], f32)[kperm]
    kyg = np.asarray(inputs["ky_norm_g"], f32)[yperm]
    kyb = np.asarray(inputs["ky_norm_b"], f32)[yperm]

    cos = np.asarray(inputs["freqs_cos"], f32)[b].T
    sin = np.asarray(inputs["freqs_sin"], f32)[b].T
    CCm = np.concatenate([cos, cos], 0)
    SSm = np.concatenate([-sin, sin], 0)
    swapP = np.zeros((128, 128), f32)
    swapP[np.arange(128), (np.arange(128) + 64) % 128] = 1.0

    xm = np.where(np.asarray(inputs["x_mask"][b]), 0.0, NEG).astype(f32)
    ym = np.where(np.asarray(inputs["y_mask"][b]), 0.0, NEG).astype(f32)
    tgv = np.tanh(np.asarray(inputs["gate"], f32)[4 * g:4 * g + 4])
    tgv = np.broadcast_to(tgv[None, :], (128, YHPC))

    wkv = np.concatenate([np.asarray(inputs["wk"], f32)[:, kperm],
                          np.asarray(inputs["wv"], f32)[:, kcols]], axis=1)
    wy = np.concatenate([np.asarray(inputs["wk_y"], f32)[:, yperm],
                         np.asarray(inputs["wv_y"], f32)[:, ycols]], axis=1)

    def hl(a, scale):
        hi, lo = _hilo(a, scale)
        return np.ascontiguousarray(np.stack([hi, lo], axis=1))

    bf = lambda a: np.ascontiguousarray(a).astype(NP16)
    return {
        "xhl": hl(x[b].T, XS), "yhl": hl(y[b].T, XS),
        "wqhl": hl(np.asarray(inputs["wq"], f32)[:, qperm], WS),
        "wkvhl": hl(wkv, WS), "wyhl": hl(wy, WS),
        "wo": bf(np.asarray(inputs["wo"], f32)[qcols, :]),
        "CC": bf(CCm), "SSp": bf(SSm), "swapP": bf(swapP),
        "qgc": np.ascontiguousarray(qg.reshape(HPC, HD).T).astype(f32),
        "kgc": np.ascontiguousarray(kg.reshape(KVPC, HD).T).astype(f32),
        "kygc": np.ascontiguousarray(kyg.reshape(YHPC, HD).T).astype(f32),
        "qb": np.ascontiguousarray(qb.reshape(HPC, HD).T).astype(f32),
        "kb": np.ascontiguousarray(kb.reshape(KVPC, HD).T).astype(f32),
        "kyb": np.ascontiguousarray(kyb.reshape(YHPC, HD).T).astype(f32),
        "xmask": np.ascontiguousarray(xm.reshape(NKC, 128).T).astype(f32),
        "ymask": np.ascontiguousarray(ym.reshape(NYKC, 128).T).astype(f32),
        "tg": np.ascontiguousarray(tgv).astype(f32),
    }


def _get_runner():
    global _RUNNER
    if _RUNNER is None:
        _RUNNER = _build_program()
    return _RUNNER


def _get_exec():
    """Build (once) a cached jitted shard_map executable for the program."""
    global _EXEC
    if _EXEC is None:
        import jax
        from jax.experimental.shard_map import shard_map
        from jax.sharding import Mesh, NamedSharding, PartitionSpec

        nc = _get_runner()
        from concourse import bass2jax as b2j
        b2j.install_neuronx_cc_hook()

        pname = (nc.partition_id_tensor.name
                 if nc.partition_id_tensor else None)
        in_names, out_names, out_avals = [], [], []
        for alloc in nc.m.functions[0].allocations:
            if not isinstance(alloc, mybir.MemoryLocationSet):
                continue
            name = alloc.memorylocations[0].name
            if alloc.kind == "ExternalInput":
                if name != pname:
                    in_names.append(name)
            elif alloc.kind == "ExternalOutput":
                out_names.append(name)
                out_avals.append(jax.core.ShapedArray(
                    tuple(alloc.tensor_shape), mybir.dt.np(alloc.dtype)))
        n_params = len(in_names)
        all_in = list(in_names + out_names)
        if pname is not None:
            all_in.append(pname)
        all_in = tuple(all_in)
        donate = tuple(range(n_params, n_params + len(out_names)))

        def _body(*args):
            operands = list(args)
            if pname is not None:
                operands.append(b2j.partition_id_tensor())
            outs = b2j._bass_exec_p.bind(
                *operands, out_avals=tuple(out_avals), in_names=all_in,
                out_names=tuple(out_names),
                lowering_input_output_aliases=(),
                sim_require_finite=True, sim_require_nnan=True, nc=nc)
            return tuple(outs)

        devices = jax.devices()[:N_CORES]
        mesh = Mesh(np.asarray(devices), ("core",))
        nin = n_params + len(out_names)
        sharded = jax.jit(
            shard_map(_body, mesh=mesh,
                      in_specs=(PartitionSpec("core"),) * nin,
                      out_specs=(PartitionSpec("core"),) * len(out_names),
                      check_rep=False),
            donate_argnums=donate, keep_unused=True)
        shd = NamedSharding(mesh, PartitionSpec("core"))
        mk0 = [jax.jit(lambda a=a: __import__("jax.numpy", fromlist=["x"]
                                              ).zeros((N_CORES * a.shape[0],)
                                                      + a.shape[1:], a.dtype),
                       out_shardings=shd) for a in out_avals]
        _EXEC = (sharded, in_names, out_names, out_avals, shd, mk0)
    return _EXEC


def _concat_inputs(in_maps):
    sharded, in_names, out_names, out_avals, shd, mk0 = _get_exec()
    return [np.concatenate([np.asarray(in_maps[c][nm])
                            for c in range(N_CORES)], axis=0)
            for nm in in_names]


def _exec(concat_in, device_put=False):
    """Run once; returns {name: full concatenated np array}."""
    import jax
    sharded, in_names, out_names, out_avals, shd, mk0 = _get_exec()
    if device_put:
        concat_in = [jax.device_put(a, shd) for a in concat_in]
    outs = sharded(*concat_in, *[f() for f in mk0])
    return dict(zip(out_names, outs))


def run_on_cores(in_maps, trace=False):
    nc = _get_runner()
    return bass_utils.run_bass_kernel_spmd(
        nc, in_maps, core_ids=list(range(N_CORES)), trace=trace)


def kernel(**inputs):
    in_maps = [_prep_core_inputs(inputs, c) for c in range(N_CORES)]
    outs = _exec(_concat_inputs(in_maps))
    o = np.asarray(outs["out"]).astype(np.float32).reshape(N_CORES, S, D)
    out = np.zeros((B, S, D), np.float32)
    for c in range(N_CORES):
        out[c // TP] += o[c]
    return out


# revision 116
# speedup vs baseline: 1.0058x; 1.0024x over previous
"""Sharded attention kernel for Trainium2 (8 NeuronCores, Bass/Tile).

Module: x->(wq,wk,wv) qk-norm + rope + GQA self-attn  (+)  gated cross-attn
over y->(wk_y,wv_y), then wo.  B=2, S=2048, D=2048, H=16, KV=8, HD=128,
YL=256, YD=1024.

Sharding: 2-way batch DP x 4-way head TP.  Core c handles batch c//4 and
head group g=c%4 (q heads 4g..4g+3, kv heads 2g..2g+1, y-heads
(4g%8)..(4g%8)+3).  wo is row-sharded; the 4 partial outputs per batch are
summed on the host.  The q/k/ky layernorms normalize over the *full* flat
head dim, so each core computes partial (sum, sumsq) stats and three small
in-kernel AllReduces (groups [[0..3],[4..7]]) produce the full-row moments.

Projections run as error-compensated fp8 (e4m3) DoubleRow matmuls: the host
splits x / y / every projection weight into hi+lo fp8 streams (same DMA
bytes as fp16) with fixed power-of-2 scales; on device each contraction
pair-chunk issues three DoubleRow matmuls (hh, hl, lh) into the same fp32
PSUM accumulation, recovering ~2^-8 effective operand precision at 3/4 the
fp16 PE cost.  Evictions fold the descale into the activation-copy scale.
Attention stays fp16: scores are computed transposed (keys on partitions)
so the key mask folds into the exp() bias and P feeds PV untransposed;
softmax skips max-subtraction; denominators accumulate on DVE and reduce
across partitions with a gpsimd partition_all_reduce (Pool engine), keeping
the softmax tail entirely off PE and PSUM.  wo is interleaved into the
second query-block's attention to fill PE while Activation runs exp.
"""
import sys

sys.path.insert(0, "/opt/trn_rl_repo")

import numpy as np
import ml_dtypes

import concourse.bass as bass  # noqa: F401
import concourse.tile as tile
from concourse import bacc, mybir, bass_isa
from concourse import bass_utils
from concourse.masks import make_identity

BF16 = mybir.dt.bfloat16
DT16 = mybir.dt.float16
F32 = mybir.dt.float32
FP8 = mybir.dt.float8e4
NPFP8 = mybir.dt.np(FP8)
NP16 = np.float16
DR = mybir.MatmulPerfMode.DoubleRow

B, S, D, H, KV, YL, YD, HD = 2, 2048, 2048, 16, 8, 256, 1024, 128
N_CORES, TP = 8, 4
HPC, KVPC, YHPC = 4, 2, 4          # q / kv / y heads per core
QW, KW, YW = HPC * HD, KVPC * HD, YHPC * HD   # 512, 256, 512 output cols
NDC, NYC = D // 128, YD // 128     # contraction chunks: 16, 8
NSB, SB = 4, 512                   # seq blocks for projections
NQB, QB = 2, 1024                  # query blocks for attention
JB = 512                           # attention j-chunk (max moving free)
NKC = S // 128                     # 16 key chunks (self)
NYKC = YL // 128                   # 2 key chunks (cross)
NST = S // 128                     # 16 seq tiles for wo
EPS_QK, EPS_KY = 1e-5, 1e-6
NEG = -1.0e30
XS, WS = 32.0, 2048.0              # fixed fp8 scales (power of 2)
DESC = 1.0 / (XS * WS)             # eviction descale

_RUNNER = None
_EXEC = None


def _build_program(use_cc=True):
    nc = bacc.Bacc("TRN2", target_bir_lowering=False, debug=False,
                   num_devices=N_CORES if use_cc else 1)

    def din(name, shape, dt=DT16):
        return nc.dram_tensor(name, shape, dt, kind="ExternalInput")

    t = dict(
        xhl=din("xhl", [D, 2, S], FP8),
        yhl=din("yhl", [YD, 2, YL], FP8),
        wqhl=din("wqhl", [D, 2, QW], FP8),
        wkvhl=din("wkvhl", [D, 2, 2 * KW], FP8),
        wyhl=din("wyhl", [YD, 2, 2 * YW], FP8),
        wo=din("wo", [QW, D]),
        CC=din("CC", [128, S]),
        SSp=din("SSp", [128, S]),
        swapP=din("swapP", [128, 128]),
        qgc=din("qgc", [128, HPC], F32),
        kgc=din("kgc", [128, KVPC], F32),
        kygc=din("kygc", [128, YHPC], F32),
        qb=din("qb", [128, HPC], F32),
        kb=din("kb", [128, KVPC], F32),
        kyb=din("kyb", [128, YHPC], F32),
        xmask=din("xmask", [128, NKC], F32),
        ymask=din("ymask", [128, NYKC], F32),
        tg=din("tg", [128, YHPC], F32),
        out=nc.dram_tensor("out", [S, D], DT16, kind="ExternalOutput"),
        kin=nc.dram_tensor("kin", [2, S], F32),
        kout=nc.dram_tensor("kout", [2, S], F32),
        kyin=nc.dram_tensor("kyin", [2, YL], F32),
        kyout=nc.dram_tensor("kyout", [2, YL], F32),
        qin=nc.dram_tensor("qin", [2, S], F32),
        qout=nc.dram_tensor("qout", [2, S], F32),
        lnr=nc.dram_tensor("lnr", [6, S], DT16),
        groups=[[0, 1, 2, 3], [4, 5, 6, 7]],
        use_cc=use_cc,
    )

    with tile.TileContext(nc) as tc:
        _emit(nc, tc, t)
    nc.compile()
    return nc


def _allreduce(nc, t, name, nh, h):
    """AllReduce of one half-major block of the partial LN stats (cc) /
    local copy (no-cc).  The stats tensors are laid out half-major:
    block h holds [sum_cols | sq_cols] for its column range contiguously,
    so the collective input is a single contiguous run."""
    Alu = mybir.AluOpType
    tin, tout = t[name + "in"], t[name + "out"]
    n = 2 * tin.shape[1] // nh
    src = bass.AP(tensor=tin.ap().tensor, offset=h * n, ap=[[1, n]])
    dst = bass.AP(tensor=tout.ap().tensor, offset=h * n, ap=[[1, n]])
    if t["use_cc"]:
        nc.gpsimd.collective_compute(
            "AllReduce", Alu.add, replica_groups=t["groups"],
            ins=[src], outs=[dst])


def _st(t, name):
    """Stats source for moments: the AllReduce output when collectives
    run; the local partials directly in the single-core timing variant
    (the collective adds no local engine work there)."""
    return t[name + "out"] if t["use_cc"] else t[name + "in"]


def _emit(nc, tc, t):
    AF = mybir.ActivationFunctionType
    Alu = mybir.AluOpType

    cm_consts = tc.tile_pool(name="consts", bufs=1)
    consts = cm_consts.__enter__()

    # ---------------- constants / small inputs ----------------
    ident = consts.tile([128, 128], DT16, tag="ident", name="ident")
    make_identity(nc, ident[:, :])
    ones_col = consts.tile([128, 1], DT16, tag="ones_col", name="ones_col")
    nc.vector.memset(ones_col[:, :], 1.0)
    ones_bf = consts.tile([128, 1], BF16, tag="ones_bf", name="ones_bf")
    nc.vector.memset(ones_bf[:, :], 1.0)
    swp = consts.tile([128, 128], DT16, tag="swp", name="swp")
    cc = consts.tile([128, S], DT16, tag="cc", name="cc")
    ssp = consts.tile([128, S], DT16, tag="ssp", name="ssp")
    qg_sb = consts.tile([128, HPC], F32, tag="qgc", name="qgc")
    nc.gpsimd.dma_start(qg_sb[:, :], t["qgc"].ap())
    kg_sb = consts.tile([128, KVPC], F32, tag="kgc", name="kgc")
    nc.gpsimd.dma_start(kg_sb[:, :], t["kgc"].ap())
    kyg_sb = consts.tile([128, YHPC], F32, tag="kygc", name="kygc")
    nc.gpsimd.dma_start(kyg_sb[:, :], t["kygc"].ap())
    qb_sb = consts.tile([128, HPC], F32, tag="qb", name="qb")
    nc.gpsimd.dma_start(qb_sb[:, :], t["qb"].ap())
    kb_sb = consts.tile([128, KVPC], F32, tag="kb", name="kb")
    nc.gpsimd.dma_start(kb_sb[:, :], t["kb"].ap())
    kyb_sb = consts.tile([128, YHPC], F32, tag="kyb", name="kyb")
    nc.gpsimd.dma_start(kyb_sb[:, :], t["kyb"].ap())
    xm_sb = consts.tile([128, NKC], F32, tag="xm", name="xm")
    nc.gpsimd.dma_start(xm_sb[:, :], t["xmask"].ap())
    ym_sb = consts.tile([128, NYKC], F32, tag="ym", name="ym")
    nc.gpsimd.dma_start(ym_sb[:, :], t["ymask"].ap())
    tg_sb = consts.tile([128, YHPC], F32, tag="tg", name="tg")
    nc.gpsimd.dma_start(tg_sb[:, :], t["tg"].ap())

    # ---------------- phase-1 pools ----------------
    cm_raw = tc.tile_pool(name="p_raw", bufs=1)
    p_raw = cm_raw.__enter__()
    cm_w = tc.tile_pool(name="p_w", bufs=1)
    p_w = cm_w.__enter__()
    cm_x = tc.tile_pool(name="p_x", bufs=1)
    p_x = cm_x.__enter__()
    cm_wsq = tc.tile_pool(name="w_sq", bufs=2)
    w_sq = cm_wsq.__enter__()
    cm_stg = tc.tile_pool(name="w_stg", bufs=2)
    w_stg = cm_stg.__enter__()
    cm_ar = tc.tile_pool(name="w_ar", bufs=2)
    w_ar = cm_ar.__enter__()

    def stat_out(tname, nh, stg, col0, blk):
        """Write the accumulated (sum | sumsq) staging row with one DMA
        into the half-major stats layout."""
        hw_ = t[tname].shape[1] // nh          # cols per half
        h, rel = col0 // hw_, col0 % hw_
        dst = bass.AP(tensor=t[tname].ap().tensor, offset=2 * h * hw_ + rel,
                      ap=[[hw_, 2], [1, blk]])
        nc.scalar.dma_start(dst, stg[:, :2 * blk])

    # hi/lo-packed fp8 tiles: dim -2 selects the stream (0=hi, 1=lo)
    wq_sb = p_w.tile([128, NDC, 2, QW], FP8, tag="wq", name="wq")
    wkv_sb = p_w.tile([128, NDC, 2, 2 * KW], FP8, tag="wkv", name="wkv")
    wy_sb = p_w.tile([128, NYC, 2, 2 * YW], FP8, tag="wy", name="wy")
    y_sb = p_w.tile([128, NYC, 2, YL], FP8, tag="y", name="y")

    xr = t["xhl"].ap().rearrange("(c p) two s -> p c two s", p=128)
    wq_r = t["wqhl"].ap().rearrange("(c p) two m -> p c two m", p=128)
    wkv_r = t["wkvhl"].ap().rearrange("(c p) two m -> p c two m", p=128)
    wy_r = t["wyhl"].ap().rearrange("(c p) two m -> p c two m", p=128)
    y_r = t["yhl"].ap().rearrange("(c p) two s -> p c two s", p=128)

    # x tiles for all 4 seq blocks stay resident (both passes read them)
    xts = [p_x.tile([128, NDC, 2, SB], FP8, tag=f"x_{sb}", name=f"x_{sb}")
           for sb in range(NSB)]

    # load order: kv pass runs first, so wkv-hi + x0-hi strips lead (the
    # hh-term sweeps start on them), then the lo strips, then the rest.
    for s in (0, 1):
        for c in range(0, NDC, 4):
            nc.sync.dma_start(wkv_sb[:, c:c + 4, s, :],
                              wkv_r[:, c:c + 4, s, :])
            nc.sync.dma_start(xts[0][:, c:c + 4, s, :],
                              xr[:, c:c + 4, s, 0:SB])
    for sb in range(1, NSB):
        for c in range(0, NDC, 8):
            for s in (0, 1):
                nc.sync.dma_start(xts[sb][:, c:c + 8, s, :],
                                  xr[:, c:c + 8, s, sb * SB:(sb + 1) * SB])
    for c in range(0, NDC, 4):
        for s in (0, 1):
            nc.sync.dma_start(wq_sb[:, c:c + 4, s, :],
                              wq_r[:, c:c + 4, s, :])
    nc.sync.dma_start(swp[:, :], t["swapP"].ap())
    nc.sync.dma_start(cc[:, :], t["CC"].ap())
    nc.sync.dma_start(ssp[:, :], t["SSp"].ap())
    for s in (0, 1):
        nc.sync.dma_start(y_sb[:, :, s, :], y_r[:, :, s, :])
    for s in (0, 1):
        nc.sync.dma_start(wy_sb[:, :, s, :], wy_r[:, :, s, :])

    # raw projection outputs (fp16); later reused in place for QT/KT/vnat
    qraw = [p_raw.tile([128, S], DT16, tag=f"qraw{i}", name=f"qraw{i}")
            for i in range(HPC)]
    kraw = [p_raw.tile([128, S], DT16, tag=f"kraw{i}", name=f"kraw{i}")
            for i in range(KVPC)]
    vraw = [p_raw.tile([128, S], DT16, tag=f"vraw{i}", name=f"vraw{i}")
            for i in range(KVPC)]
    ykraw = [p_raw.tile([128, YL], DT16, tag=f"ykraw{i}", name=f"ykraw{i}")
             for i in range(YHPC)]
    yvraw = [p_raw.tile([128, YL], DT16, tag=f"yvraw{i}", name=f"yvraw{i}")
             for i in range(YHPC)]

    cm_psA = tc.tile_pool(name="pp_projA", bufs=2, space="PSUM")
    pp_proj = cm_psA.__enter__()

    def proj_fp8(w_t, x_t, npair, col0, blk, ps):
        """3-term compensated fp8 DoubleRow accumulation into ps.
        hh terms sweep first so compute can start before lo streams land."""
        first = True
        for (ws_, xs_) in ((0, 0), (0, 1), (1, 0)):
            for c in range(npair):
                nc.tensor.matmul(
                    ps[:, :blk],
                    w_t[:, 2 * c:2 * c + 2, ws_, col0:col0 + 128],
                    x_t[:, 2 * c:2 * c + 2, xs_, :blk],
                    start=first,
                    stop=((ws_, xs_) == (1, 0) and c == npair - 1),
                    perf_mode=DR)
                first = False

    def proj_block(w_t, x_t, npair, col0, dst, sb, blk,
                   stg=None, first=False):
        ps = pp_proj.tile([128, SB], F32, tag="proj", name="proj")
        proj_fp8(w_t, x_t, npair, col0, blk, ps)
        nc.scalar.activation(dst[:, sb * blk:(sb + 1) * blk], ps[:, :blk],
                             AF.Copy, scale=DESC)
        if stg is not None:
            # LN stats off PE: Pool partition-reduces the evicted tile and
            # its square; DVE accumulates the row into the staging tile
            sq = w_sq.tile([128, SB], BF16, tag="sqscratch", name="sqscratch")
            nc.scalar.activation(sq[:, :blk], ps[:, :blk], AF.Square,
                                 scale=DESC)
            ars = w_ar.tile([128, SB], DT16, tag="ars", name="ars")
            nc.gpsimd.partition_all_reduce(
                ars[:, :blk], dst[:, sb * blk:(sb + 1) * blk],
                channels=128, reduce_op=bass_isa.ReduceOp.add)
            arq = w_ar.tile([128, SB], BF16, tag="arq", name="arq")
            nc.gpsimd.partition_all_reduce(
                arq[:, :blk], sq[:, :blk],
                channels=128, reduce_op=bass_isa.ReduceOp.add)
            if first:
                nc.vector.tensor_copy(stg[0:1, :blk], ars[0:1, :blk])
                nc.vector.tensor_copy(stg[0:1, blk:2 * blk], arq[0:1, :blk])
            else:
                nc.vector.tensor_add(stg[0:1, :blk], stg[0:1, :blk],
                                     ars[0:1, :blk])
                nc.vector.tensor_add(stg[0:1, blk:2 * blk],
                                     stg[0:1, blk:2 * blk], arq[0:1, :blk])

    cm_rm = tc.tile_pool(name="rows_m", bufs=2, side="right")
    rows_m = cm_rm.__enter__()
    cm_wln = tc.tile_pool(name="w_ln", bufs=1, side="right")
    w_ln = cm_wln.__enter__()
    cm_wln2 = tc.tile_pool(name="w_ln2", bufs=2, side="right")
    w_ln2 = cm_wln2.__enter__()

    def moments(src_t, nh, n, inv_scale, eps, length, r_rstd, col0=0,
                ncols=None):
        """src_t is half-major (sum cols | sq cols per half); process the
        half starting at column col0.  Partition-parallel math on
        [128, ncols/128]; rstd and -mu*rstd slices land in lnr rows
        (r_rstd, r_rstd+1) via one DMA each way."""
        ncols = ncols or length
        J = ncols // 128
        hw_ = length // nh

        ab = rows_m.tile([128, 2, 16], F32, tag="mab", name="mab")
        src = bass.AP(tensor=src_t.ap().tensor,
                      offset=2 * (col0 // hw_) * hw_ + col0 % hw_,
                      ap=[[J, 128], [hw_, 2], [1, J]])
        nc.scalar.dma_start(ab[:, :, :J], src)
        a, b = ab[:, 0, :J], ab[:, 1, :J]
        nc.vector.tensor_scalar_mul(a, a, inv_scale / n)
        nc.vector.tensor_scalar_mul(b, b, inv_scale / n)
        c_ = rows_m.tile([128, 16], F32, tag="mc", name="mc")
        nc.vector.tensor_mul(c_[:, :J], a, a)
        nc.vector.tensor_tensor(b, b, c_[:, :J], Alu.subtract)
        nc.vector.tensor_scalar_add(b, b, eps)
        # rsqrt via exp(-0.5*ln(var)): stays in the exp activation table
        # (no table switch before attention); the Newton step below refines.
        nc.scalar.activation(c_[:, :J], b, AF.Ln)
        nc.scalar.activation(c_[:, :J], c_[:, :J], AF.Exp, scale=-0.5)
        d = rows_m.tile([128, 16], F32, tag="md", name="md")
        nc.vector.tensor_mul(d[:, :J], c_[:, :J], c_[:, :J])
        nc.vector.tensor_mul(d[:, :J], d[:, :J], b)
        nc.vector.tensor_scalar(out=d[:, :J], in0=d[:, :J],
                                scalar1=-0.5, scalar2=1.5,
                                op0=Alu.mult, op1=Alu.add)
        nc.vector.tensor_mul(c_[:, :J], c_[:, :J], d[:, :J])
        nc.vector.tensor_mul(a, a, c_[:, :J])
        nc.vector.tensor_scalar_mul(a, a, -1.0)
        ra = rows_m.tile([128, 2, 16], DT16, tag="mra", name="mra")
        nc.vector.tensor_copy(ra[:, 0, :J], c_[:, :J])
        nc.vector.tensor_copy(ra[:, 1, :J], a)
        out_r = bass.AP(tensor=t["lnr"].ap().tensor,
                        offset=r_rstd * S + col0,
                        ap=[[J, 128], [S, 2], [1, J]])
        nc.sync.dma_start(out_r, ra[:, :, :J])

    def dma_bcast(rgng, r_rstd, col0, ncols):
        """One DMA: broadcast lnr rows (r_rstd, r_rstd+1) column slice into
        the [128, 2, *] rgng tile."""
        src_ap = bass.AP(tensor=t["lnr"].ap().tensor,
                         offset=r_rstd * S + col0,
                         ap=[[0, 128], [S, 2], [1, ncols]])
        nc.sync.dma_start(rgng[:, :, col0:col0 + ncols], src_ap)

    cm_swp = tc.tile_pool(name="pp_swap", bufs=2, space="PSUM")
    pp_swap = cm_swp.__enter__()

    # broadcast-row tiles for the three LN streams ([:, 0, :]=rstd*,
    # [:, 1, :]=-mu*rstd)
    q_rr = w_ln.tile([128, 2, S], DT16, tag="q_rr", name="q_rr")
    k_rr = w_ln.tile([128, 2, S], DT16, tag="k_rr", name="k_rr")
    ky_rr = w_ln.tile([128, 2, YL], DT16, tag="ky_rr", name="ky_rr")

    def ln_unit(raw, rr, g_col, b_col, col0, ncols, rope, ps_pool=None,
                eng="dve"):
        """LayerNorm (+optional rope) of one head's column slice, in place.
        In phase 1 the per-head gain/bias and swap eviction run on
        Activation (per-partition scalars) to unload DVE at the boundary;
        units deferred into the attention region (ps_pool set) keep those
        ops on DVE so they don't compete with exp."""
        sl = slice(col0, col0 + ncols)
        in_attn = ps_pool is not None
        t1 = w_ln2.tile([128, SB], DT16, tag="lnt1", name="lnt1")
        nc.vector.tensor_mul(t1[:, :ncols], raw[:, sl], rr[:, 0, sl])
        nc.vector.tensor_add(t1[:, :ncols], t1[:, :ncols], rr[:, 1, sl])
        if not rope:
            if eng == "act":
                nc.scalar.activation(raw[:, sl], t1[:, :ncols], AF.Identity,
                                     bias=b_col, scale=g_col)
            else:
                nc.vector.tensor_scalar(out=raw[:, sl], in0=t1[:, :ncols],
                                        scalar1=g_col, scalar2=b_col,
                                        op0=Alu.mult, op1=Alu.add)
            return
        if eng == "act":
            nc.scalar.activation(t1[:, :ncols], t1[:, :ncols], AF.Identity,
                                 bias=b_col, scale=g_col)
        else:
            nc.vector.tensor_scalar(out=t1[:, :ncols], in0=t1[:, :ncols],
                                    scalar1=g_col, scalar2=b_col,
                                    op0=Alu.mult, op1=Alu.add)
        sw = w_ln2.tile([128, SB], DT16, tag="swap", name="swap")
        pool = ps_pool or pp_swap
        ps = pool.tile([128, JB], F32, tag="tp" if ps_pool is None else "po",
                       name="lnswp")
        nc.tensor.matmul(ps[:, :ncols], swp[:, :], t1[:, :ncols],
                         start=True, stop=True)
        if eng == "act":
            nc.scalar.activation(sw[:, :ncols], ps[:, :ncols], AF.Copy)
        else:
            nc.vector.tensor_copy(sw[:, :ncols], ps[:, :ncols])
        nc.vector.tensor_mul(t1[:, :ncols], t1[:, :ncols], cc[:, sl])
        nc.vector.tensor_mul(sw[:, :ncols], sw[:, :ncols], ssp[:, sl])
        nc.vector.tensor_add(raw[:, sl], t1[:, :ncols], sw[:, :ncols])

    def vtrans(raw, c):
        tp = pp_swap.tile([128, 128], DT16, tag="vtp", name="vtp")
        nc.tensor.transpose(tp[:, :], raw[:, c * 128:(c + 1) * 128],
                            ident[:, :])
        nc.scalar.activation(raw[:, c * 128:(c + 1) * 128], tp[:, :],
                             AF.Copy)

    def q_ln_unit(h, sb, ps_pool=None, eng="dve"):
        ln_unit(qraw[h], q_rr, qg_sb[:, h:h + 1],
                qb_sb[:, h:h + 1], sb * SB, SB, True, ps_pool, eng)

    def k_ln_sb(sb):
        for i in range(KVPC):
            ln_unit(kraw[i], k_rr, kg_sb[:, i:i + 1],
                    kb_sb[:, i:i + 1], sb * SB, SB, True)

    def proj_pair(w_t, x_t, npair, col0s, dsts, sb, blk, stg, first0):
        """Two colchunks term-major: all hh sweeps first so compute can
        start before the lo fp8 streams arrive (first DMA-paced block)."""
        pss = [pp_proj.tile([128, SB], F32, tag="proj", name="proj")
               for _ in col0s]
        for ti, (ws_, xs_) in enumerate(((0, 0), (0, 1), (1, 0))):
            for j, col0 in enumerate(col0s):
                for c in range(npair):
                    nc.tensor.matmul(
                        pss[j][:, :blk],
                        w_t[:, 2 * c:2 * c + 2, ws_, col0:col0 + 128],
                        x_t[:, 2 * c:2 * c + 2, xs_, :blk],
                        start=(ti == 0 and c == 0),
                        stop=(ti == 2 and c == npair - 1),
                        perf_mode=DR)
        for j, dst in enumerate(dsts):
            nc.scalar.activation(dst[:, sb * blk:(sb + 1) * blk],
                                 pss[j][:, :blk], AF.Copy, scale=DESC)
            if stg is not None:
                sq = w_sq.tile([128, SB], BF16, tag="sqscratch",
                               name="sqscratch")
                nc.scalar.activation(sq[:, :blk], pss[j][:, :blk], AF.Square,
                                     scale=DESC)
                ars = w_ar.tile([128, SB], DT16, tag="ars", name="ars")
                nc.gpsimd.partition_all_reduce(
                    ars[:, :blk], dst[:, sb * blk:(sb + 1) * blk],
                    channels=128, reduce_op=bass_isa.ReduceOp.add)
                arq = w_ar.tile([128, SB], BF16, tag="arq", name="arq")
                nc.gpsimd.partition_all_reduce(
                    arq[:, :blk], sq[:, :blk],
                    channels=128, reduce_op=bass_isa.ReduceOp.add)
                if first0 and j == 0:
                    nc.vector.tensor_copy(stg[0:1, :blk], ars[0:1, :blk])
                    nc.vector.tensor_copy(stg[0:1, blk:2 * blk],
                                          arq[0:1, :blk])
                else:
                    nc.vector.tensor_add(stg[0:1, :blk], stg[0:1, :blk],
                                         ars[0:1, :blk])
                    nc.vector.tensor_add(stg[0:1, blk:2 * blk],
                                         stg[0:1, blk:2 * blk],
                                         arq[0:1, :blk])

    # ============ phase 1a: k/v projections + stats; AR in halves ==========
    for sb in range(NSB):
        xt = xts[sb]
        kstg = w_stg.tile([1, 2 * SB], F32, tag="stg", name="stg")
        if sb == 0:
            proj_pair(wkv_sb, xt, NDC // 2, [0, 128], kraw, sb, SB,
                      kstg, True)
            proj_pair(wkv_sb, xt, NDC // 2, [KW, KW + 128], vraw, sb, SB,
                      None, False)
        else:
            for i in range(KVPC):
                proj_block(wkv_sb, xt, NDC // 2, i * 128, kraw[i], sb, SB,
                           kstg, first=(i == 0))
            for i in range(KVPC):
                proj_block(wkv_sb, xt, NDC // 2, KW + i * 128, vraw[i],
                           sb, SB)
        stat_out("kin", 2, kstg, sb * SB, SB)
        if sb == 1:
            _allreduce(nc, t, "k", 2, 0)
        if sb == 2:
            moments(_st(t, "k"), 2, KV * HD, 1.0, EPS_QK, S, 2, 0, 2 * SB)
            dma_bcast(k_rr, 2, 0, 2 * SB)
        for i in range(KVPC):
            for c in range(4 * sb, 4 * sb + 4):
                vtrans(vraw[i], c)
        if sb == 3:
            k_ln_sb(0)
    k_ln_sb(1)
    _allreduce(nc, t, "k", 2, 1)
    moments(_st(t, "k"), 2, KV * HD, 1.0, EPS_QK, S, 2, 2 * SB, 2 * SB)
    dma_bcast(k_rr, 2, 2 * SB, 2 * SB)

    # ============ phase 1b: q projections; k-LN tail interleaved; the q
    # stats AR runs in halves so query-block-0's LN lands inside the pass ===
    for sb in range(NSB):
        xt = xts[sb]
        qstg = w_stg.tile([1, 2 * SB], F32, tag="stg", name="stg")
        for i in range(HPC):
            proj_block(wq_sb, xt, NDC // 2, i * 128, qraw[i], sb, SB,
                       qstg, first=(i == 0))
        stat_out("qin", 2, qstg, sb * SB, SB)
        if sb == 0:
            k_ln_sb(2)
        if sb == 1:
            _allreduce(nc, t, "q", 2, 0)
            k_ln_sb(3)
        if sb == 2:
            moments(_st(t, "q"), 2, H * HD, 1.0, EPS_QK, S, 0, 0, 2 * SB)
            dma_bcast(q_rr, 0, 0, 2 * SB)
        if sb == 3:
            for h in range(HPC):
                q_ln_unit(h, 0)
    for h in range(HPC):
        q_ln_unit(h, 1, eng=("act" if h % 2 == 0 else "dve"))
    _allreduce(nc, t, "q", 2, 1)
    moments(_st(t, "q"), 2, H * HD, 1.0, EPS_QK, S, 0, 2 * SB, 2 * SB)
    dma_bcast(q_rr, 0, 2 * SB, 2 * SB)

    # ---- y projections ----
    ystg = w_stg.tile([1, 2 * SB], F32, tag="stg", name="stg")
    for i in range(YHPC):
        proj_block(wy_sb, y_sb, NYC // 2, i * 128, ykraw[i], 0, YL,
                   ystg, first=(i == 0))
    for i in range(YHPC):
        proj_block(wy_sb, y_sb, NYC // 2, YW + i * 128, yvraw[i], 0, YL)
    stat_out("kyin", 1, ystg, 0, YL)

    _allreduce(nc, t, "ky", 1, 0)

    for i in range(YHPC):
        for c in range(NYKC):
            vtrans(yvraw[i], c)

    # ---- ky LN (no rope, no PE work) ----
    moments(_st(t, "ky"), 1, KV * HD, 0.5, EPS_KY, YL, 4)
    dma_bcast(ky_rr, 4, 0, YL)
    for i in range(YHPC):
        ln_unit(ykraw[i], ky_rr, kyg_sb[:, i:i + 1],
                kyb_sb[:, i:i + 1], 0, YL, False, eng="act")

    # q-LN for the query-block-1 slices is deferred into the qb0 attention
    # region (emitted inside the attention loop below)

    QT, KT, YKT = qraw, kraw, ykraw

    def vnat(i, c):
        return vraw[i][:, c * 128:(c + 1) * 128]

    def yvnat(i, c):
        return yvraw[i][:, c * 128:(c + 1) * 128]

    cm_swp.__exit__(None, None, None)
    cm_psA.__exit__(None, None, None)
    cm_ar.__exit__(None, None, None)
    cm_stg.__exit__(None, None, None)
    cm_wsq.__exit__(None, None, None)
    cm_x.__exit__(None, None, None)
    cm_w.__exit__(None, None, None)

    # ============ attention + wo ============
    cm_out = tc.tile_pool(name="p_out", bufs=1)
    p_out = cm_out.__enter__()
    outT = [p_out.tile([128, S], DT16, tag=f"outT{h}", name=f"outT{h}")
            for h in range(HPC)]
    cm_wo = tc.tile_pool(name="p_wo", bufs=1)
    p_wo = cm_wo.__enter__()
    wo_sb = p_wo.tile([128, HPC, D], DT16, tag="wo", name="wo")
    nc.sync.dma_start(wo_sb[:, :, :],
                      t["wo"].ap().rearrange("(c p) m -> p c m", p=128))
    cm_wat = tc.tile_pool(name="w_at", bufs=3)
    w_at = cm_wat.__enter__()
    cm_pt = tc.tile_pool(name="w_pt", bufs=7)
    w_pt = cm_pt.__enter__()
    cm_wob = tc.tile_pool(name="w_ob", bufs=2)
    w_ob = cm_wob.__enter__()

    cm_wops = tc.tile_pool(name="pp_wo", bufs=2, space="PSUM")
    cm_sc = tc.tile_pool(name="pp_sc", bufs=2, space="PSUM")
    cm_pv = tc.tile_pool(name="pp_pv", bufs=1, space="PSUM")
    pp_wo = cm_wops.__enter__()
    pp_sc = cm_sc.__enter__()
    pp_pv = cm_pv.__enter__()

    def attend(h, qb_i):
        """Self + gated cross attention for query block qb_i of head h."""
        q0 = qb_i * QB
        pv = pp_pv.tile([128, QB], F32, tag="pv", name="pv")

        def chunks(KT_h, vn, nkc, mask_sb, acc_tag):
            acc = w_at.tile([128, QB], DT16, tag=acc_tag, name=acc_tag)
            accB = (w_at.tile([128, QB], DT16, tag=acc_tag + "B",
                              name=acc_tag + "B") if nkc > 4 else None)
            ptA0 = ptB0 = None

            def emit_sc(c):
                sc = pp_sc.tile([128, QB], F32, tag="sc", name="sc")
                for j in range(0, QB, JB):
                    nc.tensor.matmul(sc[:, j:j + JB],
                                     KT_h[:, c * 128:(c + 1) * 128],
                                     QT[h][:, q0 + j:q0 + j + JB],
                                     start=True, stop=True)
                return sc

            # software pipeline: scores one chunk ahead so PE never
            # blocks behind exp(c) when issuing pv(c)
            sc_cur = emit_sc(0)
            for c in range(nkc):
                pt = w_pt.tile([128, QB], DT16, tag="ptile", name="ptile")
                nc.scalar.activation(pt[:, :], sc_cur[:, :], AF.Exp,
                                     bias=mask_sb[:, c:c + 1])
                if c + 1 < nkc:
                    sc_cur = emit_sc(c + 1)
                for j in range(0, QB, JB):
                    nc.tensor.matmul(pv[:, j:j + JB], vn(c),
                                     pt[:, j:j + JB],
                                     start=(c == 0), stop=(c == nkc - 1))
                # two parallel accumulation chains; each chain's first two
                # tiles fuse into one add (no initial copy)
                if accB is not None and c % 4 == 3 and c < nkc - 4:
                    if c == 3:
                        ptB0 = pt
                    elif ptB0 is not None:
                        nc.gpsimd.tensor_add(accB[:, :], ptB0[:, :], pt[:, :])
                        ptB0 = None
                    else:
                        nc.gpsimd.tensor_add(accB[:, :], accB[:, :],
                                             pt[:, :])
                elif c == 0:
                    ptA0 = pt
                elif ptA0 is not None:
                    nc.vector.tensor_add(acc[:, :], ptA0[:, :], pt[:, :])
                    ptA0 = None
                else:
                    nc.vector.tensor_add(acc[:, :], acc[:, :], pt[:, :])
            if accB is not None:
                nc.vector.tensor_add(acc[:, :], acc[:, :], accB[:, :])
            pvb = w_at.tile([128, QB], DT16, tag="pvb" + acc_tag,
                            name="pvb" + acc_tag)
            nc.vector.tensor_copy(pvb[:, :], pv[:, :])
            ar = w_at.tile([128, QB], DT16, tag="ar" + acc_tag,
                           name="ar" + acc_tag)
            nc.gpsimd.partition_all_reduce(ar[:, :], acc[:, :], channels=128,
                                           reduce_op=bass_isa.ReduceOp.add)
            with nc.allow_low_precision(reason="fp16 softmax denominators"):
                nc.vector.reciprocal(ar[:, :], ar[:, :])
            return pvb, ar

        pvbS, recS = chunks(KT[h // 2], lambda c: vnat(h // 2, c), NKC,
                            xm_sb, "S")
        pvbY, recY = chunks(YKT[h], lambda c: yvnat(h, c), NYKC, ym_sb, "Y")
        oS = w_at.tile([128, QB], DT16, tag="oS", name="oS")
        if qb_i == 0:
            nc.gpsimd.tensor_mul(oS[:, :], pvbS[:, :], recS[:, :])
        else:
            nc.vector.tensor_mul(oS[:, :], pvbS[:, :], recS[:, :])
        oY = w_at.tile([128, QB], DT16, tag="oY", name="oY")
        nc.vector.scalar_tensor_tensor(
            out=oY[:, :], in0=pvbY[:, :], scalar=tg_sb[:, h:h + 1],
            in1=recY[:, :], op0=Alu.mult, op1=Alu.mult)
        if qb_i == 1:
            nc.vector.tensor_add(outT[h][:, q0:q0 + QB], oS[:, :], oY[:, :])
        else:
            nc.gpsimd.tensor_add(outT[h][:, q0:q0 + QB], oS[:, :], oY[:, :])

    def wo_tile(st):
        ob = w_ob.tile([128, D], DT16, tag="obuf", name="obuf")
        for j in range(0, D, JB):
            ps = pp_wo.tile([128, JB], F32, tag="po", name="po")
            for dc in range(HPC):
                nc.tensor.matmul(ps[:, :], outT[dc][:, st * 128:(st + 1) * 128],
                                 wo_sb[:, dc, j:j + JB],
                                 start=(dc == 0), stop=(dc == HPC - 1))
            if (j // JB) % 2 == 0:
                nc.scalar.activation(ob[:, j:j + JB], ps[:, :], AF.Copy)
            else:
                nc.vector.tensor_copy(ob[:, j:j + JB], ps[:, :])
        nc.sync.dma_start(t["out"].ap()[st * 128:(st + 1) * 128, :],
                           ob[:, :])

    for h in range(HPC):
        attend(h, 0)
        q_ln_unit(h, 2, pp_wo)
        q_ln_unit(h, 3, pp_wo)
    wo_tile(0)
    wo_tile(1)
    for h in range(HPC):
        attend(h, 1)
        if h >= 1:
            for st in range(2 * h, 2 * h + 2):
                wo_tile(st)

    # ---- wo tail: wide PSUM tiles, evictions alternating DVE/Act ----
    cm_pv.__exit__(None, None, None)
    cm_sc.__exit__(None, None, None)
    cm_wo2 = tc.tile_pool(name="pp_wo2", bufs=3, space="PSUM")
    pp_wo2 = cm_wo2.__enter__()
    for st in range(8, 16):
        ob = w_ob.tile([128, D], DT16, tag="obuf", name="obuf")
        for half in range(2):
            j0 = half * 1024
            ps = pp_wo2.tile([128, 1024], F32, tag="po2", name="po2")
            for j in range(0, 1024, JB):
                for dc in range(HPC):
                    nc.tensor.matmul(
                        ps[:, j:j + JB],
                        outT[dc][:, st * 128:(st + 1) * 128],
                        wo_sb[:, dc, j0 + j:j0 + j + JB],
                        start=(dc == 0), stop=(dc == HPC - 1))
            if st == 15:
                # last tile: quarter-granular evictions + stores shorten the
                # final drain's critical chain
                for q4 in range(2):
                    jq = j0 + q4 * JB
                    if q4 == 0:
                        nc.vector.tensor_copy(ob[:, jq:jq + JB],
                                              ps[:, q4 * JB:(q4 + 1) * JB])
                    else:
                        nc.scalar.activation(ob[:, jq:jq + JB],
                                             ps[:, q4 * JB:(q4 + 1) * JB],
                                             AF.Copy)
                    nc.sync.dma_start(
                        t["out"].ap()[st * 128:(st + 1) * 128, jq:jq + JB],
                        ob[:, jq:jq + JB])
                continue
            if half == 0:
                nc.vector.tensor_copy(ob[:, j0:j0 + 1024], ps[:, :])
            else:
                nc.scalar.activation(ob[:, j0:j0 + 1024], ps[:, :], AF.Copy)
        if st < 15:
            nc.sync.dma_start(t["out"].ap()[st * 128:(st + 1) * 128, :],
                              ob[:, :])
    cm_wo2.__exit__(None, None, None)

    cm_wops.__exit__(None, None, None)
    cm_wob.__exit__(None, None, None)
    cm_pt.__exit__(None, None, None)
    cm_wat.__exit__(None, None, None)
    cm_wo.__exit__(None, None, None)
    cm_out.__exit__(None, None, None)
    cm_wln2.__exit__(None, None, None)
    cm_wln.__exit__(None, None, None)
    cm_rm.__exit__(None, None, None)
    cm_raw.__exit__(None, None, None)
    cm_consts.__exit__(None, None, None)


def _perm_cols(ncols):
    p = np.arange(ncols).reshape(-1, HD)
    return np.concatenate([p[:, 0::2], p[:, 1::2]], axis=1).reshape(-1)


def _hilo(a, scale):
    """Split a*scale into fp8 hi + lo streams (e4m3, |.| <= 240)."""
    sa = np.clip(a * scale, -240.0, 240.0).astype(np.float32)
    hi = sa.astype(NPFP8)
    lo = (sa - hi.astype(np.float32)).astype(NPFP8)
    return np.ascontiguousarray(hi), np.ascontiguousarray(lo)


def _prep_core_inputs(inputs, core):
    b, g = core // TP, core % TP
    f32 = np.float32
    x = np.asarray(inputs["x"], f32)
    y = np.asarray(inputs["y"], f32)

    qcols = np.arange(g * QW, (g + 1) * QW)
    kcols = np.arange(g * KW, (g + 1) * KW)
    y0 = (4 * g % 8) * HD
    ycols = np.arange(y0, y0 + YW)
    qperm = qcols[_perm_cols(QW)]
    kperm = kcols[_perm_cols(KW)]
    yperm = ycols[_perm_cols(YW)]

    scale = 1.0 / np.sqrt(HD)
    qg = (np.asarray(inputs["q_norm_g"], f32) * scale)[qperm]
    qb = (np.asarray(inputs["q_norm_b"], f32) * scale)[qperm]
    kg = np.asarray(inputs["k_norm_g"], f32)[kperm]
    kb = np.asarray(inputs["k_norm_b"], f32)[kperm]
    kyg = np.asarray(inputs["ky_norm_g"], f32)[yperm]
    kyb = np.asarray(inputs["ky_norm_b"], f32)[yperm]

    cos = np.asarray(inputs["freqs_cos"], f32)[b].T
    sin = np.asarray(inputs["freqs_sin"], f32)[b].T
    CCm = np.concatenate([cos, cos], 0)
    SSm = np.concatenate([-sin, sin], 0)
    swapP = np.zeros((128, 128), f32)
    swapP[np.arange(128), (np.arange(128) + 64) % 128] = 1.0

    xm = np.where(np.asarray(inputs["x_mask"][b]), 0.0, NEG).astype(f32)
    ym = np.where(np.asarray(inputs["y_mask"][b]), 0.0, NEG).astype(f32)
    tgv = np.tanh(np.asarray(inputs["gate"], f32)[4 * g:4 * g + 4])
    tgv = np.broadcast_to(tgv[None, :], (128, YHPC))

    wkv = np.concatenate([np.asarray(inputs["wk"], f32)[:, kperm],
                          np.asarray(inputs["wv"], f32)[:, kcols]], axis=1)
    wy = np.concatenate([np.asarray(inputs["wk_y"], f32)[:, yperm],
                         np.asarray(inputs["wv_y"], f32)[:, ycols]], axis=1)

    def hl(a, scale):
        hi, lo = _hilo(a, scale)
        return np.ascontiguousarray(np.stack([hi, lo], axis=1))

    bf = lambda a: np.ascontiguousarray(a).astype(NP16)
    return {
        "xhl": hl(x[b].T, XS), "yhl": hl(y[b].T, XS),
        "wqhl": hl(np.asarray(inputs["wq"], f32)[:, qperm], WS),
        "wkvhl": hl(wkv, WS), "wyhl": hl(wy, WS),
        "wo": bf(np.asarray(inputs["wo"], f32)[qcols, :]),
        "CC": bf(CCm), "SSp": bf(SSm), "swapP": bf(swapP),
        "qgc": np.ascontiguousarray(qg.reshape(HPC, HD).T).astype(f32),
        "kgc": np.ascontiguousarray(kg.reshape(KVPC, HD).T).astype(f32),
        "kygc": np.ascontiguousarray(kyg.reshape(YHPC, HD).T).astype(f32),
        "qb": np.ascontiguousarray(qb.reshape(HPC, HD).T).astype(f32),
        "kb": np.ascontiguousarray(kb.reshape(KVPC, HD).T).astype(f32),
        "kyb": np.ascontiguousarray(kyb.reshape(YHPC, HD).T).astype(f32),
        "xmask": np.ascontiguousarray(xm.reshape(NKC, 128).T).astype(f32),
        "ymask": np.ascontiguousarray(ym.reshape(NYKC, 128).T).astype(f32),
        "tg": np.ascontiguousarray(tgv).astype(f32),
    }


def _get_runner():
    global _RUNNER
    if _RUNNER is None:
        _RUNNER = _build_program()
    return _RUNNER


def _get_exec():
    """Build (once) a cached jitted shard_map executable for the program."""
    global _EXEC
    if _EXEC is None:
        import jax
        from jax.experimental.shard_map import shard_map
        from jax.sharding import Mesh, NamedSharding, PartitionSpec

        nc = _get_runner()
        from concourse import bass2jax as b2j
        b2j.install_neuronx_cc_hook()

        pname = (nc.partition_id_tensor.name
                 if nc.partition_id_tensor else None)
        in_names, out_names, out_avals = [], [], []
        for alloc in nc.m.functions[0].allocations:
            if not isinstance(alloc, mybir.MemoryLocationSet):
                continue
            name = alloc.memorylocations[0].name
            if alloc.kind == "ExternalInput":
                if name != pname:
                    in_names.append(name)
            elif alloc.kind == "ExternalOutput":
                out_names.append(name)
                out_avals.append(jax.core.ShapedArray(
                    tuple(alloc.tensor_shape), mybir.dt.np(alloc.dtype)))
        n_params = len(in_names)
        all_in = list(in_names + out_names)
        if pname is not None:
            all_in.append(pname)
        all_in = tuple(all_in)
        donate = tuple(range(n_params, n_params + len(out_names)))

        def _body(*args):
            operands = list(args)
            if pname is not None:
                operands.append(b2j.partition_id_tensor())
            outs = b2j._bass_exec_p.bind(
                *operands, out_avals=tuple(out_avals), in_names=all_in,
                out_names=tuple(out_names),
                lowering_input_output_aliases=(),
                sim_require_finite=True, sim_require_nnan=True, nc=nc)
            return tuple(outs)

        devices = jax.devices()[:N_CORES]
        mesh = Mesh(np.asarray(devices), ("core",))
        nin = n_params + len(out_names)
        sharded = jax.jit(
            shard_map(_body, mesh=mesh,
                      in_specs=(PartitionSpec("core"),) * nin,
                      out_specs=(PartitionSpec("core"),) * len(out_names),
                      check_rep=False),
            donate_argnums=donate, keep_unused=True)
        shd = NamedSharding(mesh, PartitionSpec("core"))
        mk0 = [jax.jit(lambda a=a: __import__("jax.numpy", fromlist=["x"]
                                              ).zeros((N_CORES * a.shape[0],)
                                                      + a.shape[1:], a.dtype),
                       out_shardings=shd) for a in out_avals]
        _EXEC = (sharded, in_names, out_names, out_avals, shd, mk0)
    return _EXEC


def _concat_inputs(in_maps):
    sharded, in_names, out_names, out_avals, shd, mk0 = _get_exec()
    return [np.concatenate([np.asarray(in_maps[c][nm])
                            for c in range(N_CORES)], axis=0)
            for nm in in_names]


def _exec(concat_in, device_put=False):
    """Run once; returns {name: full concatenated np array}."""
    import jax
    sharded, in_names, out_names, out_avals, shd, mk0 = _get_exec()
    if device_put:
        concat_in = [jax.device_put(a, shd) for a in concat_in]
    outs = sharded(*concat_in, *[f() for f in mk0])
    return dict(zip(out_names, outs))


def run_on_cores(in_maps, trace=False):
    nc = _get_runner()
    return bass_utils.run_bass_kernel_spmd(
        nc, in_maps, core_ids=list(range(N_CORES)), trace=trace)


def kernel(**inputs):
    in_maps = [_prep_core_inputs(inputs, c) for c in range(N_CORES)]
    outs = _exec(_concat_inputs(in_maps))
    o = np.asarray(outs["out"]).astype(np.float32).reshape(N_CORES, S, D)
    out = np.zeros((B, S, D), np.float32)
    for c in range(N_CORES):
        out[c // TP] += o[c]
    return out
